# revision 1
# baseline (speedup 1.0000x reference)
"""nn_CrossAttention Trainium2 kernel — 8-core SPMD Bass/Tile implementation.

Sharding: core p -> batch b = p//2, query-row half h = p%2 (data parallel over
B=4, sequence-parallel over TN within each batch pair).

Per-core dataflow:
  tT,xT   PE-transpose of the f32 inputs (feature dim onto partitions)
  qT,kT   projections in channel-major layout (f32r = tf32-class precision)
  v       projection in natural row-major layout (bf16)
  sT      score tiles computed *transposed* (keys on partitions, queries free)
  eT      exp(SCALE*s) straight from PSUM via ScalarE, cast to bf16
  D       softmax denominators via ones-vector matmul (PE partition reduce)
  oT      v.T @ eT accumulated over keys, normalized by 1/D broadcast
  AllGather within core pairs exchanges oT halves; the reference's
  "transpose(1,2).reshape" permutation then becomes *contiguous* DRAM rows
  (z-buffer), so the output projection reads it with plain DMAs.
  out     permuted-o chunks @ Wp + bp  (full TN rows; host keeps its half)
"""
from contextlib import ExitStack

import numpy as np

import concourse.bass as bass
import concourse.tile as tile
from concourse import bacc, mybir
from concourse.bass_utils import run_bass_kernel_spmd
from concourse.masks import make_identity

F32 = mybir.dt.float32
BF16 = mybir.dt.bfloat16
F32R = mybir.dt.float32r
EXP = mybir.ActivationFunctionType.Exp

B, N, TN, C = 4, 4096, 4096, 384
TNS = TN // 2
SCALE = (C // 8) ** -0.5
N_CORES = 8

QK_DT = F32R   # q/k/score path (tf32-class)
VE_DT = BF16   # v/e/o path


def build(qk_dt=QK_DT, ve_dt=VE_DT, repeat=1, with_collective=True):
    nc = bacc.Bacc("TRN2", target_bir_lowering=False, debug=False,
                   num_devices=N_CORES)
    x_d = nc.dram_tensor("x", [N, C], F32, kind="ExternalInput").ap()
    t_d = nc.dram_tensor("t", [TNS, C], F32, kind="ExternalInput").ap()
    w_d = {n: nc.dram_tensor(n, [C, C], F32, kind="ExternalInput").ap()
           for n in ("Wq", "Wk", "Wv", "Wp")}
    bp_d = nc.dram_tensor("bp", [1, C], F32, kind="ExternalInput").ap()
    out_d = nc.dram_tensor("out", [TN, C], F32, kind="ExternalOutput").ap()

    with tile.TileContext(nc) as tc:
        _kernel_body(nc, tc, x_d, t_d, w_d, bp_d, out_d, qk_dt, ve_dt,
                     repeat, with_collective)
    nc.compile()
    return nc


def _kernel_body(nc, tc, x_d, t_d, w_d, bp_d, out_d, qk_st, ve_st,
                 repeat, with_collective):
    with ExitStack() as ctx:
        consts = ctx.enter_context(tc.tile_pool(name="consts", bufs=1))
        persist = ctx.enter_context(tc.tile_pool(name="persist", bufs=1))
        dram = ctx.enter_context(tc.tile_pool(name="dram", bufs=1, space="DRAM"))

        ident = consts.tile([128, 128], F32)
        make_identity(nc, ident)
        ones_col = consts.tile([128, 1], ve_st)
        nc.vector.memset(ones_col[:], 1.0)
        ones_row = consts.tile([1, 128], F32)
        nc.vector.memset(ones_row[:], 1.0)

        w_sb = {}
        with tc.tile_pool(name="wstage", bufs=2) as wstage:
            for name in ("Wq", "Wk", "Wv", "Wp"):
                cw = persist.tile([128, 3 * C], qk_st, name=f"{name}_sb",
                                  tag=f"{name}_sb")
                for dc in range(3):
                    st = wstage.tile([128, C], F32, name="wst", tag="wst")
                    nc.sync.dma_start(st[:], w_d[name][dc * 128:(dc + 1) * 128, :])
                    nc.scalar.copy(cw[:, dc * C:(dc + 1) * C], st[:])
                w_sb[name] = cw
            bst = wstage.tile([1, C], F32, name="bst", tag="wst")
            nc.sync.dma_start(bst[:], bp_d[:])
            with tc.tile_pool(name="bpsum", bufs=1, space="PSUM") as bpsum:
                bias_ps = bpsum.tile([128, C], F32)
                nc.tensor.matmul(bias_ps[:], ones_row[:], bst[:],
                                 start=True, stop=True)
                bias_b = persist.tile([128, C], F32)
                nc.vector.tensor_copy(bias_b[:], bias_ps[:])

        def wch(name, dc, cc=None):
            if cc is None:
                return w_sb[name][:, dc * C:(dc + 1) * C]
            return w_sb[name][:, dc * C + cc * 128: dc * C + (cc + 1) * 128]

        for rep in range(repeat):
            _one_pass(nc, tc, x_d, t_d, out_d, qk_st, ve_st, ident, ones_col,
                      ones_row, wch, bias_b, dram, with_collective, rep)


def _one_pass(nc, tc, x_d, t_d, out_d, qk_st, ve_st, ident, ones_col,
              ones_row, wch, bias_b, dram, with_collective, rep):
    with tc.tile_pool(name="attin", bufs=1) as attin:
        # ---- tT -> qT ----
        with tc.tile_pool(name="tstage", bufs=3) as tstage, \
             tc.tile_pool(name="trpsum", bufs=2, space="PSUM") as trpsum:
            tT = [tstage.tile([128, TNS], qk_st, name=f"tT{dc}", tag=f"tT{dc}",
                              bufs=1) for dc in range(3)]
            for i in range(TNS // 128):
                trow = tstage.tile([128, C], F32, name="trow", tag="trow")
                nc.sync.dma_start(trow[:], t_d[i * 128:(i + 1) * 128, :])
                for dc in range(3):
                    pst = trpsum.tile([128, 128], F32, name="pst", tag="pst")
                    nc.tensor.transpose(pst[:], trow[:, dc * 128:(dc + 1) * 128],
                                        ident[:])
                    nc.vector.tensor_copy(tT[dc][:, i * 128:(i + 1) * 128], pst[:])
            qT = attin.tile([128, 3 * TNS], qk_st, name="qT", tag="qT")
            with tc.tile_pool(name="qpsum", bufs=2, space="PSUM") as qpsum:
                for cc in range(3):
                    for nt in range(TNS // 512):
                        ps = qpsum.tile([128, 512], F32, name="qps", tag="qps")
                        for dc in range(3):
                            nc.tensor.matmul(
                                ps[:], wch("Wq", dc, cc),
                                tT[dc][:, nt * 512:(nt + 1) * 512],
                                start=(dc == 0), stop=(dc == 2))
                        nc.scalar.copy(
                            qT[:, cc * TNS + nt * 512: cc * TNS + (nt + 1) * 512],
                            ps[:])

        # ---- xT -> kT, v ----
        with tc.tile_pool(name="xstage", bufs=3) as xstage, \
             tc.tile_pool(name="xtrpsum", bufs=2, space="PSUM") as xtrpsum:
            xT = [xstage.tile([128, N], qk_st, name=f"xT{dc}", tag=f"xT{dc}",
                              bufs=1) for dc in range(3)]
            for i in range(N // 128):
                xrow = xstage.tile([128, C], F32, name="xrow", tag="xrow")
                nc.sync.dma_start(xrow[:], x_d[i * 128:(i + 1) * 128, :])
                for dc in range(3):
                    pst = xtrpsum.tile([128, 128], F32, name="xpst", tag="xpst")
                    nc.tensor.transpose(pst[:], xrow[:, dc * 128:(dc + 1) * 128],
                                        ident[:])
                    nc.vector.tensor_copy(xT[dc][:, i * 128:(i + 1) * 128], pst[:])
            kT = [attin.tile([128, N], qk_st, name=f"kT{cc}", tag=f"kT{cc}")
                  for cc in range(3)]
            v_all = attin.tile([128, 32 * C], ve_st, name="v_all", tag="v_all")
            with tc.tile_pool(name="kvpsum", bufs=3, space="PSUM") as kvpsum:
                for cc in range(3):
                    for nt in range(N // 512):
                        ps = kvpsum.tile([128, 512], F32, name="kps", tag="kps")
                        for dc in range(3):
                            nc.tensor.matmul(
                                ps[:], wch("Wk", dc, cc),
                                xT[dc][:, nt * 512:(nt + 1) * 512],
                                start=(dc == 0), stop=(dc == 2))
                        nc.scalar.copy(kT[cc][:, nt * 512:(nt + 1) * 512], ps[:])
                for n32 in range(32):
                    ps = kvpsum.tile([128, C], F32, name="vps", tag="vps")
                    for dc in range(3):
                        nc.tensor.matmul(
                            ps[:], xT[dc][:, n32 * 128:(n32 + 1) * 128],
                            wch("Wv", dc),
                            start=(dc == 0), stop=(dc == 2))
                    nc.scalar.copy(v_all[:, n32 * C:(n32 + 1) * C], ps[:])

        # ---- attention (scores transposed; no max-subtraction needed) ----
        oT = [attin.tile([128, TNS], F32, name=f"oT{cc}", tag=f"oT{cc}")
              for cc in range(3)]
        with tc.tile_pool(name="spsum", bufs=3, space="PSUM") as spsum, \
             tc.tile_pool(name="opsum", bufs=1, space="PSUM") as opsum, \
             tc.tile_pool(name="dpsum", bufs=1, space="PSUM") as dpsum, \
             tc.tile_pool(name="epool", bufs=6) as epool, \
             tc.tile_pool(name="npool", bufs=2) as npool:
            for T in range(TNS // 512):
                o_ps = [opsum.tile([128, 512], F32, name=f"ops{cc}",
                                   tag=f"ops{cc}") for cc in range(3)]
                d_ps = dpsum.tile([1, 512], F32, name="dps", tag="dps")
                for n32 in range(32):
                    s_ps = spsum.tile([128, 512], F32, name="sps", tag="sps")
                    for cc in range(3):
                        nc.tensor.matmul(
                            s_ps[:], kT[cc][:, n32 * 128:(n32 + 1) * 128],
                            qT[:, cc * TNS + T * 512: cc * TNS + (T + 1) * 512],
                            start=(cc == 0), stop=(cc == 2))
                    e_t = epool.tile([128, 512], ve_st, name="e_t", tag="e_t")
                    nc.scalar.activation(e_t[:], s_ps[:], EXP, scale=SCALE)
                    for cc in range(3):
                        nc.tensor.matmul(
                            o_ps[cc][:],
                            v_all[:, n32 * C + cc * 128: n32 * C + (cc + 1) * 128],
                            e_t[:], start=(n32 == 0), stop=(n32 == 31))
                    nc.tensor.matmul(d_ps[:], ones_col[:], e_t[:],
                                     start=(n32 == 0), stop=(n32 == 31))
                rec = npool.tile([1, 512], F32, name="rec", tag="rec")
                nc.vector.reciprocal(rec[:], d_ps[:])
                b_ps = spsum.tile([128, 512], F32, name="bps", tag="sps")
                nc.tensor.matmul(b_ps[:], ones_row[:], rec[:],
                                 start=True, stop=True)
                rec_b = npool.tile([128, 512], F32, name="rec_b", tag="rec_b")
                nc.vector.tensor_copy(rec_b[:], b_ps[:])
                for cc in range(3):
                    nc.vector.tensor_mul(oT[cc][:, T * 512:(T + 1) * 512],
                                         o_ps[cc][:], rec_b[:])

        oT_d = dram.tile([C, TNS], F32, name=f"oT_d{rep}", tag="oT_d")
        for cc in range(3):
            nc.sync.dma_start(oT_d[cc * 128:(cc + 1) * 128, :], oT[cc][:])

    # ---- pair exchange + permutation-to-contiguous ----
    zbuf = dram.tile([TN, C], F32, name=f"zbuf{rep}", tag="zbuf")
    zview = zbuf[:].rearrange("a b -> (a b)").rearrange("(c t) -> c t", t=TN)
    if with_collective:
        gath = dram.tile([2 * C, TNS], F32, name=f"gath{rep}", tag="gath")
        nc.gpsimd.collective_compute(
            "AllGather", mybir.AluOpType.bypass,
            replica_groups=[[0, 1], [2, 3], [4, 5], [6, 7]],
            ins=[oT_d[:].opt()], outs=[gath[:].opt()])
        for g in range(2):
            nc.sync.dma_start(zview[:, g * TNS:(g + 1) * TNS],
                              gath[g * C:(g + 1) * C, :])
    else:
        for g in range(2):
            nc.sync.dma_start(zview[:, g * TNS:(g + 1) * TNS], oT_d[:])

    # ---- permuted output projection ----
    with tc.tile_pool(name="fpool", bufs=3) as fpool, \
         tc.tile_pool(name="fpsum", bufs=2, space="PSUM") as fpsum, \
         tc.tile_pool(name="ftpsum", bufs=2, space="PSUM") as ftpsum:
        for it in range(TN // 128):
            r_t = fpool.tile([128, C], F32, name="r_t", tag="r_t")
            nc.sync.dma_start(r_t[:], zbuf[it * 128:(it + 1) * 128, :])
            op_ch = fpool.tile([128, 3 * 128], qk_st, name="op_ch", tag="op_ch")
            for jc in range(3):
                p_tr = ftpsum.tile([128, 128], F32, name="p_tr", tag="p_tr")
                nc.tensor.transpose(p_tr[:], r_t[:, jc * 128:(jc + 1) * 128],
                                    ident[:])
                nc.vector.tensor_copy(op_ch[:, jc * 128:(jc + 1) * 128], p_tr[:])
            out_ps = fpsum.tile([128, C], F32, name="out_ps", tag="out_ps")
            for jc in range(3):
                nc.tensor.matmul(out_ps[:], op_ch[:, jc * 128:(jc + 1) * 128],
                                 wch("Wp", jc), start=(jc == 0), stop=(jc == 2))
            o_t = fpool.tile([128, C], F32, name="o_t", tag="o_t")
            nc.vector.tensor_add(o_t[:], out_ps[:], bias_b[:])
            nc.sync.dma_start(out_d[it * 128:(it + 1) * 128, :], o_t[:])


def make_in_maps(inputs):
    x = np.asarray(inputs["x"], np.float32)
    t = np.asarray(inputs["t"], np.float32)
    maps = []
    for p in range(N_CORES):
        b, h = p // 2, p % 2
        maps.append({
            "x": np.ascontiguousarray(x[b]),
            "t": np.ascontiguousarray(t[b, h * TNS:(h + 1) * TNS]),
            "Wq": np.asarray(inputs["Wq"], np.float32),
            "Wk": np.asarray(inputs["Wk"], np.float32),
            "Wv": np.asarray(inputs["Wv"], np.float32),
            "Wp": np.asarray(inputs["Wp"], np.float32),
            "bp": np.asarray(inputs["bp"], np.float32).reshape(1, C),
        })
    return maps


def assemble(results):
    out = np.empty((B, TN, C), np.float32)
    for p in range(N_CORES):
        b, h = p // 2, p % 2
        out[b, h * TNS:(h + 1) * TNS] = results[p]["out"][h * TNS:(h + 1) * TNS]
    return out


_NC_CACHE = {}


def _get_nc(repeat=1):
    key = repeat
    if key not in _NC_CACHE:
        _NC_CACHE[key] = build(repeat=repeat)
    return _NC_CACHE[key]


def kernel(**inputs) -> np.ndarray:
    nc = _get_nc()
    in_maps = make_in_maps(inputs)
    res = run_bass_kernel_spmd(nc, in_maps, list(range(N_CORES)))
    return assemble(res.results)



# revision 7
# speedup vs baseline: 1.5273x; 1.5273x over previous
"""nn_CrossAttention Trainium2 kernel — 8-core SPMD Bass/Tile implementation.

Sharding: core p -> batch b = p//2, query-row half h = p%2 (data parallel over
B=4, sequence-parallel over TN within each batch pair).

Per-core dataflow (v2 — pipelined ReduceScatter exchange):
  tT,xT   PE-transpose of the f32 inputs (feature dim onto partitions)
  qT,kT   projections in channel-major layout (f32r = tf32-class precision)
  v       projection in natural row-major layout (bf16)
  sT      score tiles computed *transposed* (keys on partitions, queries free)
  eT      exp(SCALE*s) straight from PSUM via ScalarE, cast to bf16
  D       softmax denominators via ones-vector matmul (PE partition reduce)

  Exchange: the reference's "transpose(1,2).reshape" permutation sends
  channels [192h,192h+192) x all TN queries to pair-member h. Each core
  writes its normalized o tiles into a [384, 1024] bf16 buffer twice —
  columns [0:512] scaled by (1-h)/D and [512:1024] by h/D (h delivered as a
  per-core 0/1 input folded into the reciprocal-broadcast matmul) — and a
  pairwise ReduceScatter(add) then yields exactly the [192, 1024] slab this
  core needs: zeros from my masked half + partner's data. Chunked per
  512-query tile so the collectives overlap the attention compute.

  out     each core projects only its own TN/2 output rows (permuted rows
          are contiguous in the assembled [192, TN] buffer) @ Wp + bp.
"""
from contextlib import ExitStack

import numpy as np

import concourse.bass as bass
import concourse.tile as tile
from concourse import bacc, mybir
from concourse.bass_utils import run_bass_kernel_spmd
from concourse.masks import make_identity

F32 = mybir.dt.float32
BF16 = mybir.dt.bfloat16
F32R = mybir.dt.float32r
EXP = mybir.ActivationFunctionType.Exp

B, N, TN, C = 4, 4096, 4096, 384
TNS = TN // 2
H = C // 2          # channels per pair-member after the permutation split
SCALE = (C // 8) ** -0.5
N_CORES = 8
QT = 512            # query-tile (chunk) width
NQT = TNS // QT     # chunks per core

QK_DT = F32R   # q/k/score path (tf32-class)
VE_DT = BF16   # v/e/o path


def build(qk_dt=QK_DT, ve_dt=VE_DT, repeat=1, with_collective=True):
    nc = bacc.Bacc("TRN2", target_bir_lowering=False, debug=False,
                   num_devices=N_CORES)
    x_d = nc.dram_tensor("x", [N, C], F32, kind="ExternalInput").ap()
    t_d = nc.dram_tensor("t", [TNS, C], F32, kind="ExternalInput").ap()
    w_d = {n: nc.dram_tensor(n, [C, C], F32, kind="ExternalInput").ap()
           for n in ("Wq", "Wk", "Wv", "Wp")}
    bp_d = nc.dram_tensor("bp", [1, C], F32, kind="ExternalInput").ap()
    hA_d = nc.dram_tensor("hselA", [1, 128], BF16, kind="ExternalInput").ap()
    hB_d = nc.dram_tensor("hselB", [1, 128], BF16, kind="ExternalInput").ap()
    out_d = nc.dram_tensor("out", [TNS, C], F32, kind="ExternalOutput").ap()

    with tile.TileContext(nc) as tc:
        _kernel_body(nc, tc, x_d, t_d, w_d, bp_d, hA_d, hB_d, out_d,
                     qk_dt, ve_dt, repeat, with_collective)
    nc.compile()
    return nc


def _kernel_body(nc, tc, x_d, t_d, w_d, bp_d, hA_d, hB_d, out_d, qk_st, ve_st,
                 repeat, with_collective):
    with ExitStack() as ctx:
        consts = ctx.enter_context(tc.tile_pool(name="consts", bufs=1))
        persist = ctx.enter_context(tc.tile_pool(name="persist", bufs=1))
        dram = ctx.enter_context(tc.tile_pool(name="dram", bufs=1, space="DRAM"))

        ident = consts.tile([128, 128], F32)
        make_identity(nc, ident)
        ident_h = consts.tile([128, 128], BF16)
        nc.vector.tensor_copy(ident_h[:], ident[:])
        ones_col = consts.tile([128, 1], ve_st)
        nc.vector.memset(ones_col[:], 1.0)
        ones_row = consts.tile([1, 128], F32)
        nc.vector.memset(ones_row[:], 1.0)
        hselA = consts.tile([1, 128], BF16)
        nc.sync.dma_start(hselA[:], hA_d[:])
        hselB = consts.tile([1, 128], BF16)
        nc.sync.dma_start(hselB[:], hB_d[:])

        w_sb = {}
        with tc.tile_pool(name="wstage", bufs=2) as wstage:
            for name in ("Wq", "Wk", "Wv", "Wp"):
                cw = persist.tile([128, 3 * C], qk_st, name=f"{name}_sb",
                                  tag=f"{name}_sb")
                for dc in range(3):
                    st = wstage.tile([128, C], F32, name="wst", tag="wst")
                    nc.sync.dma_start(st[:], w_d[name][dc * 128:(dc + 1) * 128, :])
                    nc.scalar.copy(cw[:, dc * C:(dc + 1) * C], st[:])
                w_sb[name] = cw
            bst = wstage.tile([1, C], F32, name="bst", tag="wst")
            nc.sync.dma_start(bst[:], bp_d[:])
            with tc.tile_pool(name="bpsum", bufs=1, space="PSUM") as bpsum:
                bias_ps = bpsum.tile([128, C], F32)
                nc.tensor.matmul(bias_ps[:], ones_row[:], bst[:],
                                 start=True, stop=True)
                bias_b = persist.tile([128, C], F32)
                nc.vector.tensor_copy(bias_b[:], bias_ps[:])

        def wch(name, dc, cc=None):
            if cc is None:
                return w_sb[name][:, dc * C:(dc + 1) * C]
            return w_sb[name][:, dc * C + cc * 128: dc * C + (cc + 1) * 128]

        for rep in range(repeat):
            _one_pass(nc, tc, x_d, t_d, out_d, qk_st, ve_st, ident, ident_h,
                      ones_col, hselA, hselB, wch, bias_b, dram,
                      with_collective, rep)


def _one_pass(nc, tc, x_d, t_d, out_d, qk_st, ve_st, ident, ident_h,
              ones_col, hselA, hselB, wch, bias_b, dram, with_collective, rep):
    with ExitStack() as octx:
        attin = octx.enter_context(tc.tile_pool(name="attin", bufs=1))
        # ---- tT -> qT ----
        with tc.tile_pool(name="tstage", bufs=3) as tstage, \
             tc.tile_pool(name="trpsum", bufs=2, space="PSUM") as trpsum:
            tT = [tstage.tile([128, TNS], qk_st, name=f"tT{dc}", tag=f"tT{dc}",
                              bufs=1) for dc in range(3)]
            for i in range(TNS // 128):
                trow = tstage.tile([128, C], F32, name="trow", tag="trow")
                nc.sync.dma_start(trow[:], t_d[i * 128:(i + 1) * 128, :])
                for dc in range(3):
                    pst = trpsum.tile([128, 128], F32, name="pst", tag="pst")
                    nc.tensor.transpose(pst[:], trow[:, dc * 128:(dc + 1) * 128],
                                        ident[:])
                    nc.vector.tensor_copy(tT[dc][:, i * 128:(i + 1) * 128], pst[:])
            qT = attin.tile([128, 3 * TNS], qk_st, name="qT", tag="qT")
            with tc.tile_pool(name="qpsum", bufs=2, space="PSUM") as qpsum:
                for cc in range(3):
                    for nt in range(TNS // 512):
                        ps = qpsum.tile([128, 512], F32, name="qps", tag="qps")
                        for dc in range(3):
                            nc.tensor.matmul(
                                ps[:], wch("Wq", dc, cc),
                                tT[dc][:, nt * 512:(nt + 1) * 512],
                                start=(dc == 0), stop=(dc == 2))
                        nc.scalar.copy(
                            qT[:, cc * TNS + nt * 512: cc * TNS + (nt + 1) * 512],
                            ps[:])

        # ---- xT -> kT, v ----
        with tc.tile_pool(name="xstage", bufs=3) as xstage, \
             tc.tile_pool(name="xtrpsum", bufs=2, space="PSUM") as xtrpsum:
            xT = [xstage.tile([128, N], qk_st, name=f"xT{dc}", tag=f"xT{dc}",
                              bufs=1) for dc in range(3)]
            for i in range(N // 128):
                xrow = xstage.tile([128, C], F32, name="xrow", tag="xrow")
                nc.sync.dma_start(xrow[:], x_d[i * 128:(i + 1) * 128, :])
                for dc in range(3):
                    pst = xtrpsum.tile([128, 128], F32, name="xpst", tag="xpst")
                    nc.tensor.transpose(pst[:], xrow[:, dc * 128:(dc + 1) * 128],
                                        ident[:])
                    nc.vector.tensor_copy(xT[dc][:, i * 128:(i + 1) * 128], pst[:])
            kT = [attin.tile([128, N], qk_st, name=f"kT{cc}", tag=f"kT{cc}")
                  for cc in range(3)]
            v_all = attin.tile([128, 32 * C], ve_st, name="v_all", tag="v_all")
            with tc.tile_pool(name="kvpsum", bufs=3, space="PSUM") as kvpsum:
                for cc in range(3):
                    for nt in range(N // 512):
                        ps = kvpsum.tile([128, 512], F32, name="kps", tag="kps")
                        for dc in range(3):
                            nc.tensor.matmul(
                                ps[:], wch("Wk", dc, cc),
                                xT[dc][:, nt * 512:(nt + 1) * 512],
                                start=(dc == 0), stop=(dc == 2))
                        nc.scalar.copy(kT[cc][:, nt * 512:(nt + 1) * 512], ps[:])
                for n32 in range(32):
                    ps = kvpsum.tile([128, C], F32, name="vps", tag="vps")
                    for dc in range(3):
                        nc.tensor.matmul(
                            ps[:], xT[dc][:, n32 * 128:(n32 + 1) * 128],
                            wch("Wv", dc),
                            start=(dc == 0), stop=(dc == 2))
                    nc.scalar.copy(v_all[:, n32 * C:(n32 + 1) * C], ps[:])

        # ---- attention + pipelined pair exchange ----
        # myz assembles this core's [H, TN] slab of the permuted o^T.
        myz = dram.tile([H, TN], ve_st, name=f"myz{rep}", tag="myz", bufs=2)
        rs_out = []
        with tc.tile_pool(name="spsum", bufs=3, space="PSUM") as spsum, \
             tc.tile_pool(name="opsum", bufs=1, space="PSUM") as opsum, \
             tc.tile_pool(name="dpsum", bufs=1, space="PSUM") as dpsum, \
             tc.tile_pool(name="epool", bufs=6) as epool, \
             tc.tile_pool(name="npool", bufs=2) as npool, \
             tc.tile_pool(name="mpool", bufs=6) as mpool, \
             tc.tile_pool(name="rsdram", bufs=1, space="DRAM") as rsdram:
            for T in range(NQT):
                o_ps = [opsum.tile([128, QT], F32, name=f"ops{cc}",
                                   tag=f"ops{cc}") for cc in range(3)]
                d_ps = dpsum.tile([1, QT], F32, name="dps", tag="dps")
                for n32 in range(32):
                    s_ps = spsum.tile([128, QT], F32, name="sps", tag="sps")
                    for cc in range(3):
                        nc.tensor.matmul(
                            s_ps[:], kT[cc][:, n32 * 128:(n32 + 1) * 128],
                            qT[:, cc * TNS + T * QT: cc * TNS + (T + 1) * QT],
                            start=(cc == 0), stop=(cc == 2))
                    e_t = epool.tile([128, QT], ve_st, name="e_t", tag="e_t")
                    nc.scalar.activation(e_t[:], s_ps[:], EXP, scale=SCALE)
                    for cc in range(3):
                        nc.tensor.matmul(
                            o_ps[cc][:],
                            v_all[:, n32 * C + cc * 128: n32 * C + (cc + 1) * 128],
                            e_t[:], start=(n32 == 0), stop=(n32 == 31))
                    nc.tensor.matmul(d_ps[:], ones_col[:], e_t[:],
                                     start=(n32 == 0), stop=(n32 == 31))
                rec = npool.tile([1, QT], BF16, name="rec", tag="rec")
                with nc.allow_low_precision(reason="1/D in bf16: 2^-9 rel err ok"):
                    nc.vector.reciprocal(rec[:], d_ps[:])
                # masked reciprocal broadcasts: (1-h)/D and h/D
                bA_ps = spsum.tile([128, QT], F32, name="bAps", tag="sps")
                nc.tensor.matmul(bA_ps[:], hselA[:], rec[:], start=True, stop=True)
                recA = npool.tile([128, QT], F32, name="recA", tag="recA")
                nc.vector.tensor_copy(recA[:], bA_ps[:])
                bB_ps = spsum.tile([128, QT], F32, name="bBps", tag="sps")
                nc.tensor.matmul(bB_ps[:], hselB[:], rec[:], start=True, stop=True)
                recB = npool.tile([128, QT], F32, name="recB", tag="recB")
                nc.vector.tensor_copy(recB[:], bB_ps[:])

                rs_in = rsdram.tile([2 * H, 2 * QT], ve_st, name=f"rsin{T}",
                                    tag="rsin", bufs=2)
                for cc in range(3):
                    m_t = mpool.tile([128, 2 * QT], ve_st, name="m_t", tag="m_t")
                    nc.vector.tensor_mul(m_t[:, 0:QT], o_ps[cc][:], recA[:])
                    nc.vector.tensor_mul(m_t[:, QT:2 * QT], o_ps[cc][:], recB[:])
                    nc.sync.dma_start(rs_in[cc * 128:(cc + 1) * 128, :], m_t[:])

                if with_collective:
                    ro = rsdram.tile([H, 2 * QT], ve_st, name=f"rsout{T}",
                                     tag="rsout", bufs=2)
                    nc.gpsimd.collective_compute(
                        "ReduceScatter", mybir.AluOpType.add,
                        replica_groups=[[0, 1], [2, 3], [4, 5], [6, 7]],
                        ins=[rs_in[:].opt()], outs=[ro[:].opt()])
                    nc.sync.dma_start(myz[:, T * QT:(T + 1) * QT], ro[:, 0:QT])
                    nc.sync.dma_start(myz[:, TNS + T * QT: TNS + (T + 1) * QT],
                                      ro[:, QT:2 * QT])
                else:
                    # debug path: pretend partner's half equals ours
                    nc.sync.dma_start(myz[:, T * QT:(T + 1) * QT],
                                      rs_in[0:H, 0:QT])
                    nc.sync.dma_start(myz[:, TNS + T * QT: TNS + (T + 1) * QT],
                                      rs_in[0:H, QT:2 * QT])

    # ---- permuted output projection (own half only: TNS rows) ----
    # myz.flat viewed as [TNS, C] IS this core's slice of the permuted o.
    zr = myz[:].rearrange("a b -> (a b)").rearrange("(r c) -> r c", c=C)
    with tc.tile_pool(name="fpool", bufs=3) as fpool, \
         tc.tile_pool(name="fpsum", bufs=2, space="PSUM") as fpsum, \
         tc.tile_pool(name="ftpsum", bufs=2, space="PSUM") as ftpsum:
        for it in range(TNS // 128):
            r_t = fpool.tile([128, C], ve_st, name="r_t", tag="r_t")
            nc.sync.dma_start(r_t[:], zr[it * 128:(it + 1) * 128, :])
            op_ch = fpool.tile([128, 3 * 128], qk_st, name="op_ch", tag="op_ch")
            for jc in range(3):
                p_tr = ftpsum.tile([128, 128], BF16, name="p_tr", tag="p_tr")
                nc.tensor.transpose(p_tr[:], r_t[:, jc * 128:(jc + 1) * 128],
                                    ident_h[:])
                nc.vector.tensor_copy(op_ch[:, jc * 128:(jc + 1) * 128], p_tr[:])
            out_ps = fpsum.tile([128, C], F32, name="out_ps", tag="out_ps")
            for jc in range(3):
                nc.tensor.matmul(out_ps[:], op_ch[:, jc * 128:(jc + 1) * 128],
                                 wch("Wp", jc), start=(jc == 0), stop=(jc == 2))
            o_t = fpool.tile([128, C], F32, name="o_t", tag="o_t")
            nc.vector.tensor_add(o_t[:], out_ps[:], bias_b[:])
            nc.sync.dma_start(out_d[it * 128:(it + 1) * 128, :], o_t[:])


def make_in_maps(inputs):
    x = np.asarray(inputs["x"], np.float32)
    t = np.asarray(inputs["t"], np.float32)
    import ml_dtypes
    maps = []
    for p in range(N_CORES):
        b, h = p // 2, p % 2
        maps.append({
            "x": np.ascontiguousarray(x[b]),
            "t": np.ascontiguousarray(t[b, h * TNS:(h + 1) * TNS]),
            "Wq": np.asarray(inputs["Wq"], np.float32),
            "Wk": np.asarray(inputs["Wk"], np.float32),
            "Wv": np.asarray(inputs["Wv"], np.float32),
            "Wp": np.asarray(inputs["Wp"], np.float32),
            "bp": np.asarray(inputs["bp"], np.float32).reshape(1, C),
            "hselA": np.full((1, 128), 1.0 - h, ml_dtypes.bfloat16),
            "hselB": np.full((1, 128), float(h), ml_dtypes.bfloat16),
        })
    return maps


def assemble(results):
    out = np.empty((B, TN, C), np.float32)
    for p in range(N_CORES):
        b, h = p // 2, p % 2
        out[b, h * TNS:(h + 1) * TNS] = results[p]["out"]
    return out


_NC_CACHE = {}


def _get_nc(repeat=1):
    key = repeat
    if key not in _NC_CACHE:
        _NC_CACHE[key] = build(repeat=repeat)
    return _NC_CACHE[key]


def kernel(**inputs) -> np.ndarray:
    nc = _get_nc()
    in_maps = make_in_maps(inputs)
    res = run_bass_kernel_spmd(nc, in_maps, list(range(N_CORES)))
    return assemble(res.results)


# revision 12
# speedup vs baseline: 1.5960x; 1.0450x over previous
"""nn_CrossAttention Trainium2 kernel — 8-core SPMD Bass/Tile implementation.

Sharding: core p -> batch b = p//2, query-row half h = p%2 (data parallel over
B=4, sequence-parallel over TN within each batch pair).

Per-core dataflow (v4 — bf16 datapath, XBAR DMA transposes everywhere,
pipelined ReduceScatter exchange, small-tail chunk schedule):
  x/t/weights are pre-cast to bf16 on the host; tT/xT land channel-major in
  SBUF via hardware DMA-transpose issued in 512-row slices alternating
  between the two HWDGE queues (sync/scalar), so PE projections start as
  soon as the first slices arrive.
  qT,kT   projections in channel-major layout (bf16)
  v       projection in natural row-major layout (bf16)
  sT      score tiles computed *transposed* (keys on partitions, queries free)
  eT      exp(SCALE*s) straight from PSUM via ScalarE, cast to bf16
  D       softmax denominators via ones-vector matmul (PE partition reduce)

  Exchange: the reference's "transpose(1,2).reshape" permutation sends
  channels [192h,192h+192) x all TN queries to pair-member h. Each core
  writes its normalized o tiles into a [384, 2*qw] bf16 buffer twice —
  columns [0:qw] scaled by (1-h)/D and [qw:2qw] by h/D (h delivered as a
  per-core 0/1 input folded into the reciprocal-broadcast matmul) — and a
  pairwise ReduceScatter(add) then yields exactly the [192, 2*qw] slab this
  core needs: zeros from my masked half + partner's data. Chunked per
  query tile ([512,512,512,256,256]) so the collectives overlap the
  attention compute and the last exposed collective is small.

  out     each core projects only its own TN/2 output rows; the permuted
          rows are contiguous in the assembled [192, TN] buffer, which is
          re-transposed channel-major by XBAR DMA and hit with plain
          matmuls against Wp (+bp).
"""
from contextlib import ExitStack

import numpy as np

import concourse.bass as bass
import concourse.tile as tile
from concourse import bacc, mybir
from concourse.bass_utils import run_bass_kernel_spmd
from concourse.masks import make_identity

F32 = mybir.dt.float32
BF16 = mybir.dt.bfloat16
EXP = mybir.ActivationFunctionType.Exp

B, N, TN, C = 4, 4096, 4096, 384
TNS = TN // 2
H = C // 2          # channels per pair-member after the permutation split
SCALE = (C // 8) ** -0.5
N_CORES = 8
CHUNKS = (512, 512, 512, 256, 256)  # small tail hides the RS
assert sum(CHUNKS) == TNS


def build(repeat=1, with_collective=True):
    nc = bacc.Bacc("TRN2", target_bir_lowering=False, debug=False,
                   num_devices=N_CORES)
    xb_d = nc.dram_tensor("xb", [N, C], BF16, kind="ExternalInput").ap()
    tb_d = nc.dram_tensor("tb", [TNS, C], BF16, kind="ExternalInput").ap()
    w_d = {n: nc.dram_tensor(n, [C, C], BF16, kind="ExternalInput").ap()
           for n in ("Wq", "Wk", "Wv", "Wp")}
    bp_d = nc.dram_tensor("bp", [1, C], F32, kind="ExternalInput").ap()
    hA_d = nc.dram_tensor("hselA", [1, 128], BF16, kind="ExternalInput").ap()
    hB_d = nc.dram_tensor("hselB", [1, 128], BF16, kind="ExternalInput").ap()
    out_d = nc.dram_tensor("out", [TNS, C], F32, kind="ExternalOutput").ap()

    with tile.TileContext(nc) as tc:
        _kernel_body(nc, tc, xb_d, tb_d, w_d, bp_d, hA_d, hB_d, out_d,
                     repeat, with_collective)
    nc.compile()
    return nc


def _kernel_body(nc, tc, xb_d, tb_d, w_d, bp_d, hA_d, hB_d, out_d,
                 repeat, with_collective):
    with ExitStack() as ctx:
        consts = ctx.enter_context(tc.tile_pool(name="consts", bufs=1))
        persist = ctx.enter_context(tc.tile_pool(name="persist", bufs=1))
        dram = ctx.enter_context(tc.tile_pool(name="dram", bufs=1, space="DRAM"))

        w_sb = {}
        for name in ("Wq", "Wk", "Wv", "Wp"):
            w_sb[name] = persist.tile([128, 3 * C], BF16, name=f"{name}_sb",
                                      tag=f"{name}_sb")

        def load_w(name, eng):
            for dc in range(3):
                eng.dma_start(w_sb[name][:, dc * C:(dc + 1) * C],
                              w_d[name][dc * 128:(dc + 1) * 128, :])

        ones_col = consts.tile([128, 1], BF16)
        nc.vector.memset(ones_col[:], 1.0)
        ones_row = consts.tile([1, 128], F32)
        nc.vector.memset(ones_row[:], 1.0)
        hselA = consts.tile([1, 128], BF16)
        nc.scalar.dma_start(hselA[:], hA_d[:])
        hselB = consts.tile([1, 128], BF16)
        nc.scalar.dma_start(hselB[:], hB_d[:])

        bst = consts.tile([1, C], F32)
        nc.sync.dma_start(bst[:], bp_d[:])
        with tc.tile_pool(name="bpsum", bufs=1, space="PSUM") as bpsum:
            bias_ps = bpsum.tile([128, C], F32)
            nc.tensor.matmul(bias_ps[:], ones_row[:], bst[:],
                             start=True, stop=True)
            bias_b = persist.tile([128, C], F32)
            nc.vector.tensor_copy(bias_b[:], bias_ps[:])

        def wch(name, dc, cc=None):
            if cc is None:
                return w_sb[name][:, dc * C:(dc + 1) * C]
            return w_sb[name][:, dc * C + cc * 128: dc * C + (cc + 1) * 128]

        for rep in range(repeat):
            _one_pass(nc, tc, xb_d, tb_d, out_d, ones_col,
                      hselA, hselB, wch, bias_b, dram, with_collective, rep,
                      load_w if rep == 0 else None)


def _one_pass(nc, tc, xb_d, tb_d, out_d, ones_col,
              hselA, hselB, wch, bias_b, dram, with_collective, rep,
              load_w=None):
    qeng = [nc.sync, nc.sync]
    with ExitStack() as octx:
        attin = octx.enter_context(tc.tile_pool(name="attin", bufs=1))
        # ---- tT (XBAR transpose, 512-row slices) -> qT ----
        with tc.tile_pool(name="tstage", bufs=1) as tstage:
            tT = [tstage.tile([128, TNS], BF16, name=f"tT{dc}", tag=f"tT{dc}")
                  for dc in range(3)]
            nc.sync.dma_start_transpose(tT[0][:], tb_d[:, 0:128])
            nc.sync.dma_start_transpose(tT[1][:], tb_d[:, 128:256])
            if load_w:
                load_w("Wq", nc.sync)
                load_w("Wk", nc.scalar)
            nc.sync.dma_start_transpose(tT[2][:], tb_d[:, 256:384])
            if load_w:
                load_w("Wv", nc.sync)
            qT = attin.tile([128, 3 * TNS], BF16, name="qT", tag="qT")
            with tc.tile_pool(name="qpsum", bufs=2, space="PSUM") as qpsum:
                for nt in range(TNS // 512):
                    for cc in range(3):
                        ps = qpsum.tile([128, 512], F32, name="qps", tag="qps")
                        for dc in range(3):
                            nc.tensor.matmul(
                                ps[:], wch("Wq", dc, cc),
                                tT[dc][:, nt * 512:(nt + 1) * 512],
                                start=(dc == 0), stop=(dc == 2))
                        nc.scalar.copy(
                            qT[:, cc * TNS + nt * 512: cc * TNS + (nt + 1) * 512],
                            ps[:])

        # ---- xT (XBAR transpose, 512-row slices) -> kT, v ----
        with tc.tile_pool(name="xstage", bufs=1) as xstage:
            xTc = [[xstage.tile([128, 2048], BF16, name=f"xT{dc}_{hf}",
                                tag=f"xT{dc}_{hf}") for hf in range(2)]
                   for dc in range(3)]
            qi = 0
            for hf in range(2):
                for dc in range(3):
                    qeng[qi % 2].dma_start_transpose(
                        xTc[dc][hf][:],
                        xb_d[hf * 2048:(hf + 1) * 2048,
                             dc * 128:(dc + 1) * 128])
                    qi += 1
                    if load_w and hf == 0 and dc == 0:
                        load_w("Wp", nc.scalar)
            kTc = [[attin.tile([128, 512], BF16, name=f"kT{cc}_{nt}",
                               tag=f"kT{cc}_{nt}") for nt in range(N // 512)]
                   for cc in range(3)]
            v_n = [attin.tile([128, C], BF16, name=f"v{n32}", tag=f"v{n32}")
                   for n32 in range(32)]
            with tc.tile_pool(name="kvpsum", bufs=3, space="PSUM") as kvpsum:
                for nt in range(N // 512):
                    hf, xo = nt // 4, (nt % 4) * 512
                    for cc in range(3):
                        ps = kvpsum.tile([128, 512], F32, name="kps", tag="kps")
                        for dc in range(3):
                            nc.tensor.matmul(
                                ps[:], wch("Wk", dc, cc),
                                xTc[dc][hf][:, xo:xo + 512],
                                start=(dc == 0), stop=(dc == 2))
                        nc.scalar.copy(kTc[cc][nt][:], ps[:])
                    for j in range(4):
                        n32 = nt * 4 + j
                        ps = kvpsum.tile([128, C], F32, name="vps", tag="vps")
                        for dc in range(3):
                            nc.tensor.matmul(
                                ps[:],
                                xTc[dc][hf][:, xo + j * 128: xo + (j + 1) * 128],
                                wch("Wv", dc),
                                start=(dc == 0), stop=(dc == 2))
                        nc.scalar.copy(v_n[n32][:], ps[:])

        # ---- attention + pipelined pair exchange ----
        # myz assembles this core's [H, TN] slab of the permuted o^T.
        myz = dram.tile([H, TN], BF16, name=f"myz{rep}", tag="myz", bufs=2)
        with tc.tile_pool(name="spsum", bufs=3, space="PSUM") as spsum, \
             tc.tile_pool(name="opsum", bufs=1, space="PSUM") as opsum, \
             tc.tile_pool(name="dpsum", bufs=1, space="PSUM") as dpsum, \
             tc.tile_pool(name="epool", bufs=6) as epool, \
             tc.tile_pool(name="npool", bufs=2) as npool, \
             tc.tile_pool(name="mpool", bufs=6) as mpool, \
             tc.tile_pool(name="rsdram", bufs=1, space="DRAM") as rsdram:
            q0 = 0
            for T, qw in enumerate(CHUNKS):
                o_ps = [opsum.tile([128, 512], F32, name=f"ops{cc}",
                                   tag=f"ops{cc}")[:, 0:qw] for cc in range(3)]
                d_ps = dpsum.tile([1, 512], F32, name="dps", tag="dps")[:, 0:qw]
                for n32 in range(32):
                    s_ps = spsum.tile([128, 512], F32, name="sps",
                                      tag="sps")[:, 0:qw]
                    for cc in range(3):
                        nc.tensor.matmul(
                            s_ps,
                            kTc[cc][n32 // 4][:, (n32 % 4) * 128:
                                              (n32 % 4 + 1) * 128],
                            qT[:, cc * TNS + q0: cc * TNS + q0 + qw],
                            start=(cc == 0), stop=(cc == 2))
                    e_t = epool.tile([128, 512], BF16, name="e_t",
                                     tag="e_t")[:, 0:qw]
                    nc.scalar.activation(e_t, s_ps, EXP, scale=SCALE)
                    for cc in range(3):
                        nc.tensor.matmul(
                            o_ps[cc], v_n[n32][:, cc * 128:(cc + 1) * 128],
                            e_t, start=(n32 == 0), stop=(n32 == 31))
                    nc.tensor.matmul(d_ps, ones_col[:], e_t,
                                     start=(n32 == 0), stop=(n32 == 31))
                rec = npool.tile([1, 512], BF16, name="rec", tag="rec")[:, 0:qw]
                with nc.allow_low_precision(reason="1/D in bf16: 2^-9 ok"):
                    nc.vector.reciprocal(rec, d_ps)
                # masked reciprocal broadcasts: (1-h)/D and h/D
                bA_ps = spsum.tile([128, 512], F32, name="bAps",
                                   tag="sps")[:, 0:qw]
                nc.tensor.matmul(bA_ps, hselA[:], rec, start=True, stop=True)
                recA = npool.tile([128, 512], F32, name="recA",
                                  tag="recA")[:, 0:qw]
                nc.scalar.copy(recA, bA_ps)
                bB_ps = spsum.tile([128, 512], F32, name="bBps",
                                   tag="sps")[:, 0:qw]
                nc.tensor.matmul(bB_ps, hselB[:], rec, start=True, stop=True)
                recB = npool.tile([128, 512], F32, name="recB",
                                  tag="recB")[:, 0:qw]
                nc.scalar.copy(recB, bB_ps)

                rs_in = rsdram.tile([2 * H, 2 * qw], BF16, name=f"rsin{T}",
                                    tag=f"rsin{qw}", bufs=2)
                for cc in range(3):
                    m_t = mpool.tile([128, 1024], BF16, name="m_t",
                                     tag="m_t")[:, 0:2 * qw]
                    nc.vector.tensor_mul(m_t[:, 0:qw], o_ps[cc], recA)
                    nc.vector.tensor_mul(m_t[:, qw:2 * qw], o_ps[cc], recB)
                    nc.sync.dma_start(rs_in[cc * 128:(cc + 1) * 128, :], m_t)

                if with_collective:
                    ro = rsdram.tile([H, 2 * qw], BF16, name=f"rsout{T}",
                                     tag=f"rsout{qw}", bufs=2)
                    nc.gpsimd.collective_compute(
                        "ReduceScatter", mybir.AluOpType.add,
                        replica_groups=[[0, 1], [2, 3], [4, 5], [6, 7]],
                        ins=[rs_in[:].opt()], outs=[ro[:].opt()])
                    nc.sync.dma_start(myz[:, q0:q0 + qw], ro[:, 0:qw])
                    nc.sync.dma_start(myz[:, TNS + q0: TNS + q0 + qw],
                                      ro[:, qw:2 * qw])
                else:
                    # debug path: pretend partner's half equals ours
                    nc.sync.dma_start(myz[:, q0:q0 + qw], rs_in[0:H, 0:qw])
                    nc.sync.dma_start(myz[:, TNS + q0: TNS + q0 + qw],
                                      rs_in[0:H, qw:2 * qw])
                q0 += qw

    # ---- permuted output projection (own half only: TNS rows) ----
    # myz.flat viewed as [TNS, C] IS this core's slice of the permuted o;
    # XBAR-transpose it back to channel-major and hit it with Wp.
    zr = myz[:].rearrange("a b -> (a b)").rearrange("(r c) -> r c", c=C)
    with tc.tile_pool(name="fpool", bufs=4) as fpool, \
         tc.tile_pool(name="rtpool", bufs=1) as rtpool, \
         tc.tile_pool(name="fpsum", bufs=4, space="PSUM") as fpsum:
        rTc = [[rtpool.tile([128, 1024], BF16, name=f"rT{jc}_{hf}",
                            tag=f"rT{jc}_{hf}") for hf in range(2)]
               for jc in range(3)]
        qi = 0
        for hf in range(2):
            for jc in range(3):
                qeng[qi % 2].dma_start_transpose(
                    rTc[jc][hf][:],
                    zr[hf * 1024:(hf + 1) * 1024, jc * 128:(jc + 1) * 128])
                qi += 1
        for it in range(TNS // 128):
            hf, ri = it // 8, it % 8
            out_ps = fpsum.tile([128, C], F32, name="out_ps", tag="out_ps")
            for jc in range(3):
                nc.tensor.matmul(
                    out_ps[:], rTc[jc][hf][:, ri * 128:(ri + 1) * 128],
                    wch("Wp", jc), start=(jc == 0), stop=(jc == 2))
            o_t = fpool.tile([128, C], F32, name="o_t", tag="o_t")
            nc.vector.tensor_add(o_t[:], out_ps[:], bias_b[:])
            nc.sync.dma_start(out_d[it * 128:(it + 1) * 128, :], o_t[:])


def make_in_maps(inputs):
    import ml_dtypes
    x = np.asarray(inputs["x"], ml_dtypes.bfloat16)
    t = np.asarray(inputs["t"], ml_dtypes.bfloat16)
    ws = {n: np.asarray(inputs[n], ml_dtypes.bfloat16)
          for n in ("Wq", "Wk", "Wv", "Wp")}
    maps = []
    for p in range(N_CORES):
        b, h = p // 2, p % 2
        maps.append({
            "xb": np.ascontiguousarray(x[b]),
            "tb": np.ascontiguousarray(t[b, h * TNS:(h + 1) * TNS]),
            **ws,
            "bp": np.asarray(inputs["bp"], np.float32).reshape(1, C),
            "hselA": np.full((1, 128), 1.0 - h, ml_dtypes.bfloat16),
            "hselB": np.full((1, 128), float(h), ml_dtypes.bfloat16),
        })
    return maps


def assemble(results):
    out = np.empty((B, TN, C), np.float32)
    for p in range(N_CORES):
        b, h = p // 2, p % 2
        out[b, h * TNS:(h + 1) * TNS] = results[p]["out"]
    return out


_NC_CACHE = {}


def _get_nc(repeat=1):
    key = repeat
    if key not in _NC_CACHE:
        _NC_CACHE[key] = build(repeat=repeat)
    return _NC_CACHE[key]


def kernel(**inputs) -> np.ndarray:
    nc = _get_nc()
    in_maps = make_in_maps(inputs)
    res = run_bass_kernel_spmd(nc, in_maps, list(range(N_CORES)))
    return assemble(res.results)


# revision 13
# speedup vs baseline: 1.6109x; 1.0094x over previous
"""nn_CrossAttention Trainium2 kernel — 8-core SPMD Bass/Tile implementation.

Sharding: core p -> batch b = p//2, query-row half h = p%2 (data parallel over
B=4, sequence-parallel over TN within each batch pair).

Per-core dataflow (v4 — bf16 datapath, XBAR DMA transposes everywhere,
pipelined ReduceScatter exchange, small-tail chunk schedule):
  x/t/weights are pre-cast to bf16 on the host; tT/xT land channel-major in
  SBUF via hardware DMA-transpose issued in 512-row slices alternating
  between the two HWDGE queues (sync/scalar), so PE projections start as
  soon as the first slices arrive.
  qT,kT   projections in channel-major layout (bf16)
  v       projection in natural row-major layout (bf16)
  sT      score tiles computed *transposed* (keys on partitions, queries free)
  eT      exp(SCALE*s) straight from PSUM via ScalarE, cast to bf16
  D       softmax denominators via ones-vector matmul (PE partition reduce)

  Exchange: the reference's "transpose(1,2).reshape" permutation sends
  channels [192h,192h+192) x all TN queries to pair-member h. Each core
  writes its normalized o tiles into a [384, 2*qw] bf16 buffer twice —
  columns [0:qw] scaled by (1-h)/D and [qw:2qw] by h/D (h delivered as a
  per-core 0/1 input folded into the reciprocal-broadcast matmul) — and a
  pairwise ReduceScatter(add) then yields exactly the [192, 2*qw] slab this
  core needs: zeros from my masked half + partner's data. Chunked per
  query tile ([512,512,512,256,256]) so the collectives overlap the
  attention compute and the last exposed collective is small.

  out     each core projects only its own TN/2 output rows; the permuted
          rows are contiguous in the assembled [192, TN] buffer, which is
          re-transposed channel-major by XBAR DMA and hit with plain
          matmuls against Wp (+bp).
"""
from contextlib import ExitStack

import numpy as np

import concourse.bass as bass
import concourse.tile as tile
from concourse import bacc, mybir
from concourse.bass_utils import run_bass_kernel_spmd
from concourse.masks import make_identity

F32 = mybir.dt.float32
BF16 = mybir.dt.bfloat16
EXP = mybir.ActivationFunctionType.Exp

B, N, TN, C = 4, 4096, 4096, 384
TNS = TN // 2
H = C // 2          # channels per pair-member after the permutation split
SCALE = (C // 8) ** -0.5
N_CORES = 8
CHUNKS = (512, 512, 512, 256, 256)  # small tail hides the RS
assert sum(CHUNKS) == TNS


def build(repeat=1, with_collective=True):
    nc = bacc.Bacc("TRN2", target_bir_lowering=False, debug=False,
                   num_devices=N_CORES)
    xb_d = nc.dram_tensor("xb", [N, C], BF16, kind="ExternalInput").ap()
    tb_d = nc.dram_tensor("tb", [TNS, C], BF16, kind="ExternalInput").ap()
    w_d = {n: nc.dram_tensor(n, [C, C], BF16, kind="ExternalInput").ap()
           for n in ("Wq", "Wk", "Wv", "Wp")}
    bp_d = nc.dram_tensor("bp", [1, C], F32, kind="ExternalInput").ap()
    hA_d = nc.dram_tensor("hselA", [1, 128], BF16, kind="ExternalInput").ap()
    hB_d = nc.dram_tensor("hselB", [1, 128], BF16, kind="ExternalInput").ap()
    out_d = nc.dram_tensor("out", [TNS, C], F32, kind="ExternalOutput").ap()

    with tile.TileContext(nc) as tc:
        _kernel_body(nc, tc, xb_d, tb_d, w_d, bp_d, hA_d, hB_d, out_d,
                     repeat, with_collective)
    nc.compile()
    return nc


def _kernel_body(nc, tc, xb_d, tb_d, w_d, bp_d, hA_d, hB_d, out_d,
                 repeat, with_collective):
    with ExitStack() as ctx:
        consts = ctx.enter_context(tc.tile_pool(name="consts", bufs=1))
        persist = ctx.enter_context(tc.tile_pool(name="persist", bufs=1))
        dram = ctx.enter_context(tc.tile_pool(name="dram", bufs=1, space="DRAM"))

        w_sb = {}
        for name in ("Wq", "Wk", "Wv", "Wp"):
            w_sb[name] = persist.tile([128, 3 * C], BF16, name=f"{name}_sb",
                                      tag=f"{name}_sb")

        def load_w(name, eng):
            for dc in range(3):
                eng.dma_start(w_sb[name][:, dc * C:(dc + 1) * C],
                              w_d[name][dc * 128:(dc + 1) * 128, :])

        ones_col = consts.tile([128, 1], BF16)
        nc.vector.memset(ones_col[:], 1.0)
        ones_row = consts.tile([1, 128], F32)
        nc.vector.memset(ones_row[:], 1.0)
        hselA = consts.tile([1, 128], BF16)
        nc.scalar.dma_start(hselA[:], hA_d[:])
        hselB = consts.tile([1, 128], BF16)
        nc.scalar.dma_start(hselB[:], hB_d[:])

        bst = consts.tile([1, C], F32)
        nc.sync.dma_start(bst[:], bp_d[:])
        with tc.tile_pool(name="bpsum", bufs=1, space="PSUM") as bpsum:
            bias_ps = bpsum.tile([128, C], F32)
            nc.tensor.matmul(bias_ps[:], ones_row[:], bst[:],
                             start=True, stop=True)
            bias_b = persist.tile([128, C], F32)
            nc.vector.tensor_copy(bias_b[:], bias_ps[:])

        def wch(name, dc, cc=None):
            if cc is None:
                return w_sb[name][:, dc * C:(dc + 1) * C]
            return w_sb[name][:, dc * C + cc * 128: dc * C + (cc + 1) * 128]

        for rep in range(repeat):
            _one_pass(nc, tc, xb_d, tb_d, out_d, ones_col,
                      hselA, hselB, wch, bias_b, dram, with_collective, rep,
                      load_w if rep == 0 else None)


def _one_pass(nc, tc, xb_d, tb_d, out_d, ones_col,
              hselA, hselB, wch, bias_b, dram, with_collective, rep,
              load_w=None):
    qeng = [nc.sync, nc.sync]
    with ExitStack() as octx:
        attin = octx.enter_context(tc.tile_pool(name="attin", bufs=2))
        # ---- tT (XBAR transpose, 512-row slices) -> qT ----
        with tc.tile_pool(name="tstage", bufs=1) as tstage:
            tT = [tstage.tile([128, TNS], BF16, name=f"tT{dc}", tag=f"tT{dc}")
                  for dc in range(3)]
            nc.sync.dma_start_transpose(tT[0][:], tb_d[:, 0:128])
            nc.sync.dma_start_transpose(tT[1][:], tb_d[:, 128:256])
            if load_w:
                load_w("Wq", nc.sync)
                load_w("Wk", nc.scalar)
            nc.sync.dma_start_transpose(tT[2][:], tb_d[:, 256:384])
            if load_w:
                load_w("Wv", nc.sync)
            qT = attin.tile([128, 3 * TNS], BF16, name="qT", tag="qT")
            with tc.tile_pool(name="qpsum", bufs=2, space="PSUM") as qpsum:
                for nt in range(TNS // 512):
                    for cc in range(3):
                        ps = qpsum.tile([128, 512], F32, name="qps", tag="qps")
                        for dc in range(3):
                            nc.tensor.matmul(
                                ps[:], wch("Wq", dc, cc),
                                tT[dc][:, nt * 512:(nt + 1) * 512],
                                start=(dc == 0), stop=(dc == 2))
                        nc.scalar.copy(
                            qT[:, cc * TNS + nt * 512: cc * TNS + (nt + 1) * 512],
                            ps[:])

        # ---- xT (XBAR transpose, 512-row slices) -> kT, v ----
        with tc.tile_pool(name="xstage", bufs=1) as xstage:
            xTc = [[xstage.tile([128, 2048], BF16, name=f"xT{dc}_{hf}",
                                tag=f"xT{dc}_{hf}") for hf in range(2)]
                   for dc in range(3)]
            qi = 0
            for hf in range(2):
                for dc in range(3):
                    qeng[qi % 2].dma_start_transpose(
                        xTc[dc][hf][:],
                        xb_d[hf * 2048:(hf + 1) * 2048,
                             dc * 128:(dc + 1) * 128])
                    qi += 1
                    if load_w and hf == 0 and dc == 0:
                        load_w("Wp", nc.scalar)
            kTc = [[attin.tile([128, 512], BF16, name=f"kT{cc}_{nt}",
                               tag=f"kT{cc}_{nt}") for nt in range(N // 512)]
                   for cc in range(3)]
            v_n = [attin.tile([128, C], BF16, name=f"v{n32}", tag=f"v{n32}")
                   for n32 in range(32)]
            with tc.tile_pool(name="kvpsum", bufs=3, space="PSUM") as kvpsum:
                for nt in range(N // 512):
                    hf, xo = nt // 4, (nt % 4) * 512
                    for cc in range(3):
                        ps = kvpsum.tile([128, 512], F32, name="kps", tag="kps")
                        for dc in range(3):
                            nc.tensor.matmul(
                                ps[:], wch("Wk", dc, cc),
                                xTc[dc][hf][:, xo:xo + 512],
                                start=(dc == 0), stop=(dc == 2))
                        nc.scalar.copy(kTc[cc][nt][:], ps[:])
                    for j in range(4):
                        n32 = nt * 4 + j
                        ps = kvpsum.tile([128, C], F32, name="vps", tag="vps")
                        for dc in range(3):
                            nc.tensor.matmul(
                                ps[:],
                                xTc[dc][hf][:, xo + j * 128: xo + (j + 1) * 128],
                                wch("Wv", dc),
                                start=(dc == 0), stop=(dc == 2))
                        nc.scalar.copy(v_n[n32][:], ps[:])

        # ---- attention + pipelined pair exchange ----
        # myz assembles this core's [H, TN] slab of the permuted o^T.
        myz = dram.tile([H, TN], BF16, name=f"myz{rep}", tag="myz", bufs=2)
        with tc.tile_pool(name="spsum", bufs=3, space="PSUM") as spsum, \
             tc.tile_pool(name="opsum", bufs=1, space="PSUM") as opsum, \
             tc.tile_pool(name="dpsum", bufs=1, space="PSUM") as dpsum, \
             tc.tile_pool(name="epool", bufs=6) as epool, \
             tc.tile_pool(name="npool", bufs=2) as npool, \
             tc.tile_pool(name="mpool", bufs=6) as mpool, \
             tc.tile_pool(name="rsdram", bufs=1, space="DRAM") as rsdram:
            q0 = 0
            for T, qw in enumerate(CHUNKS):
                o_ps = [opsum.tile([128, 512], F32, name=f"ops{cc}",
                                   tag=f"ops{cc}")[:, 0:qw] for cc in range(3)]
                d_ps = dpsum.tile([1, 512], F32, name="dps", tag="dps")[:, 0:qw]
                for n32 in range(32):
                    s_ps = spsum.tile([128, 512], F32, name="sps",
                                      tag="sps")[:, 0:qw]
                    for cc in range(3):
                        nc.tensor.matmul(
                            s_ps,
                            kTc[cc][n32 // 4][:, (n32 % 4) * 128:
                                              (n32 % 4 + 1) * 128],
                            qT[:, cc * TNS + q0: cc * TNS + q0 + qw],
                            start=(cc == 0), stop=(cc == 2))
                    e_t = epool.tile([128, 512], BF16, name="e_t",
                                     tag="e_t")[:, 0:qw]
                    nc.scalar.activation(e_t, s_ps, EXP, scale=SCALE)
                    for cc in range(3):
                        nc.tensor.matmul(
                            o_ps[cc], v_n[n32][:, cc * 128:(cc + 1) * 128],
                            e_t, start=(n32 == 0), stop=(n32 == 31))
                    nc.tensor.matmul(d_ps, ones_col[:], e_t,
                                     start=(n32 == 0), stop=(n32 == 31))
                rec = npool.tile([1, 512], BF16, name="rec", tag="rec")[:, 0:qw]
                with nc.allow_low_precision(reason="1/D in bf16: 2^-9 ok"):
                    nc.vector.reciprocal(rec, d_ps)
                # masked reciprocal broadcasts: (1-h)/D and h/D
                bA_ps = spsum.tile([128, 512], F32, name="bAps",
                                   tag="sps")[:, 0:qw]
                nc.tensor.matmul(bA_ps, hselA[:], rec, start=True, stop=True)
                recA = npool.tile([128, 512], F32, name="recA",
                                  tag="recA")[:, 0:qw]
                nc.scalar.copy(recA, bA_ps)
                bB_ps = spsum.tile([128, 512], F32, name="bBps",
                                   tag="sps")[:, 0:qw]
                nc.tensor.matmul(bB_ps, hselB[:], rec, start=True, stop=True)
                recB = npool.tile([128, 512], F32, name="recB",
                                  tag="recB")[:, 0:qw]
                nc.scalar.copy(recB, bB_ps)

                rs_in = rsdram.tile([2 * H, 2 * qw], BF16, name=f"rsin{T}",
                                    tag=f"rsin{qw}", bufs=2)
                for cc in range(3):
                    m_t = mpool.tile([128, 1024], BF16, name="m_t",
                                     tag="m_t")[:, 0:2 * qw]
                    nc.vector.tensor_mul(m_t[:, 0:qw], o_ps[cc], recA)
                    nc.vector.tensor_mul(m_t[:, qw:2 * qw], o_ps[cc], recB)
                    nc.sync.dma_start(rs_in[cc * 128:(cc + 1) * 128, :], m_t)

                if with_collective:
                    ro = rsdram.tile([H, 2 * qw], BF16, name=f"rsout{T}",
                                     tag=f"rsout{qw}", bufs=2)
                    nc.gpsimd.collective_compute(
                        "ReduceScatter", mybir.AluOpType.add,
                        replica_groups=[[0, 1], [2, 3], [4, 5], [6, 7]],
                        ins=[rs_in[:].opt()], outs=[ro[:].opt()])
                    nc.sync.dma_start(myz[:, q0:q0 + qw], ro[:, 0:qw])
                    nc.sync.dma_start(myz[:, TNS + q0: TNS + q0 + qw],
                                      ro[:, qw:2 * qw])
                else:
                    # debug path: pretend partner's half equals ours
                    nc.sync.dma_start(myz[:, q0:q0 + qw], rs_in[0:H, 0:qw])
                    nc.sync.dma_start(myz[:, TNS + q0: TNS + q0 + qw],
                                      rs_in[0:H, qw:2 * qw])
                q0 += qw

    # ---- permuted output projection (own half only: TNS rows) ----
    # myz.flat viewed as [TNS, C] IS this core's slice of the permuted o;
    # XBAR-transpose it back to channel-major and hit it with Wp.
    zr = myz[:].rearrange("a b -> (a b)").rearrange("(r c) -> r c", c=C)
    with tc.tile_pool(name="fpool", bufs=4) as fpool, \
         tc.tile_pool(name="rtpool", bufs=1) as rtpool, \
         tc.tile_pool(name="fpsum", bufs=4, space="PSUM") as fpsum:
        rTc = [[rtpool.tile([128, 1024], BF16, name=f"rT{jc}_{hf}",
                            tag=f"rT{jc}_{hf}") for hf in range(2)]
               for jc in range(3)]
        qi = 0
        for hf in range(2):
            for jc in range(3):
                qeng[qi % 2].dma_start_transpose(
                    rTc[jc][hf][:],
                    zr[hf * 1024:(hf + 1) * 1024, jc * 128:(jc + 1) * 128])
                qi += 1
        for it in range(TNS // 128):
            hf, ri = it // 8, it % 8
            out_ps = fpsum.tile([128, C], F32, name="out_ps", tag="out_ps")
            for jc in range(3):
                nc.tensor.matmul(
                    out_ps[:], rTc[jc][hf][:, ri * 128:(ri + 1) * 128],
                    wch("Wp", jc), start=(jc == 0), stop=(jc == 2))
            o_t = fpool.tile([128, C], F32, name="o_t", tag="o_t")
            nc.vector.tensor_add(o_t[:], out_ps[:], bias_b[:])
            nc.sync.dma_start(out_d[it * 128:(it + 1) * 128, :], o_t[:])


def make_in_maps(inputs):
    import ml_dtypes
    x = np.asarray(inputs["x"], ml_dtypes.bfloat16)
    t = np.asarray(inputs["t"], ml_dtypes.bfloat16)
    ws = {n: np.asarray(inputs[n], ml_dtypes.bfloat16)
          for n in ("Wq", "Wk", "Wv", "Wp")}
    maps = []
    for p in range(N_CORES):
        b, h = p // 2, p % 2
        maps.append({
            "xb": np.ascontiguousarray(x[b]),
            "tb": np.ascontiguousarray(t[b, h * TNS:(h + 1) * TNS]),
            **ws,
            "bp": np.asarray(inputs["bp"], np.float32).reshape(1, C),
            "hselA": np.full((1, 128), 1.0 - h, ml_dtypes.bfloat16),
            "hselB": np.full((1, 128), float(h), ml_dtypes.bfloat16),
        })
    return maps


def assemble(results):
    out = np.empty((B, TN, C), np.float32)
    for p in range(N_CORES):
        b, h = p // 2, p % 2
        out[b, h * TNS:(h + 1) * TNS] = results[p]["out"]
    return out


_NC_CACHE = {}


def _get_nc(repeat=1):
    key = repeat
    if key not in _NC_CACHE:
        _NC_CACHE[key] = build(repeat=repeat)
    return _NC_CACHE[key]


def kernel(**inputs) -> np.ndarray:
    nc = _get_nc()
    in_maps = make_in_maps(inputs)
    res = run_bass_kernel_spmd(nc, in_maps, list(range(N_CORES)))
    return assemble(res.results)


# revision 16
# speedup vs baseline: 1.7432x; 1.0821x over previous
"""nn_CrossAttention Trainium2 kernel — 8-core SPMD Bass/Tile implementation.

Sharding: core p -> batch b = p//2, query-row half h = p%2 (data parallel over
B=4, sequence-parallel over TN within each batch pair).

Per-core dataflow (v4 — bf16 datapath, XBAR DMA transposes everywhere,
pipelined ReduceScatter exchange, small-tail chunk schedule):
  x/t/weights are pre-cast to bf16 on the host; tT/xT land channel-major in
  SBUF via hardware DMA-transpose issued in 512-row slices alternating
  between the two HWDGE queues (sync/scalar), so PE projections start as
  soon as the first slices arrive.
  qT,kT   projections in channel-major layout (bf16)
  v       projection in natural row-major layout (bf16)
  sT      score tiles computed *transposed* (keys on partitions, queries free)
  eT      exp(SCALE*s) straight from PSUM via ScalarE, cast to bf16
  D       softmax denominators via ones-vector matmul (PE partition reduce)

  Exchange: the reference's "transpose(1,2).reshape" permutation sends
  channels [192h,192h+192) x all TN queries to pair-member h. Each core
  writes its normalized o tiles into a [384, 2*qw] bf16 buffer twice —
  columns [0:qw] scaled by (1-h)/D and [qw:2qw] by h/D (h delivered as a
  per-core 0/1 input folded into the reciprocal-broadcast matmul) — and a
  pairwise ReduceScatter(add) then yields exactly the [192, 2*qw] slab this
  core needs: zeros from my masked half + partner's data. Chunked per
  query tile ([512,512,512,256,256]) so the collectives overlap the
  attention compute and the last exposed collective is small.

  out     each core projects only its own TN/2 output rows; the permuted
          rows are contiguous in the assembled [192, TN] buffer, which is
          re-transposed channel-major by XBAR DMA and hit with plain
          matmuls against Wp (+bp).
"""
from contextlib import ExitStack

import numpy as np

import concourse.bass as bass
import concourse.tile as tile
from concourse import bacc, mybir
from concourse.bass_utils import run_bass_kernel_spmd
from concourse.masks import make_identity

F32 = mybir.dt.float32
BF16 = mybir.dt.bfloat16
EXP = mybir.ActivationFunctionType.Exp

B, N, TN, C = 4, 4096, 4096, 384
TNS = TN // 2
H = C // 2          # channels per pair-member after the permutation split
SCALE = (C // 8) ** -0.5
N_CORES = 8
CHUNKS = (512, 512, 512, 256, 256)  # small tail hides the RS
assert sum(CHUNKS) == TNS


def build(repeat=1, with_collective=True):
    nc = bacc.Bacc("TRN2", target_bir_lowering=False, debug=False,
                   num_devices=N_CORES)
    xb_d = nc.dram_tensor("xb", [N, C], BF16, kind="ExternalInput").ap()
    tb_d = nc.dram_tensor("tb", [TNS, C], BF16, kind="ExternalInput").ap()
    w_d = {n: nc.dram_tensor(n, [C, C], BF16, kind="ExternalInput").ap()
           for n in ("Wq", "Wk", "Wv", "Wp")}
    bp_d = nc.dram_tensor("bp", [1, C], F32, kind="ExternalInput").ap()
    hA_d = nc.dram_tensor("hselA", [1, 128], BF16, kind="ExternalInput").ap()
    hB_d = nc.dram_tensor("hselB", [1, 128], BF16, kind="ExternalInput").ap()
    out_d = nc.dram_tensor("out", [TNS, C], F32, kind="ExternalOutput").ap()

    with tile.TileContext(nc) as tc:
        _kernel_body(nc, tc, xb_d, tb_d, w_d, bp_d, hA_d, hB_d, out_d,
                     repeat, with_collective)
    nc.compile()
    return nc


def _kernel_body(nc, tc, xb_d, tb_d, w_d, bp_d, hA_d, hB_d, out_d,
                 repeat, with_collective):
    with ExitStack() as ctx:
        consts = ctx.enter_context(tc.tile_pool(name="consts", bufs=1))
        persist = ctx.enter_context(tc.tile_pool(name="persist", bufs=1))
        dram = ctx.enter_context(tc.tile_pool(name="dram", bufs=1, space="DRAM"))

        w_sb = {}
        for name in ("Wq", "Wk", "Wv", "Wp"):
            w_sb[name] = persist.tile([128, 3 * C], BF16, name=f"{name}_sb",
                                      tag=f"{name}_sb")

        def load_w(name, eng):
            for dc in range(3):
                eng.dma_start(w_sb[name][:, dc * C:(dc + 1) * C],
                              w_d[name][dc * 128:(dc + 1) * 128, :])

        ones_col = consts.tile([128, 1], BF16)
        nc.vector.memset(ones_col[:], 1.0)
        ones_row = consts.tile([1, 128], F32)
        nc.vector.memset(ones_row[:], 1.0)
        hselA = consts.tile([1, 128], BF16)
        nc.scalar.dma_start(hselA[:], hA_d[:])
        hselB = consts.tile([1, 128], BF16)
        nc.scalar.dma_start(hselB[:], hB_d[:])

        bst = consts.tile([1, C], F32)
        nc.sync.dma_start(bst[:], bp_d[:])
        with tc.tile_pool(name="bpsum", bufs=1, space="PSUM") as bpsum:
            bias_ps = bpsum.tile([128, C], F32)
            nc.tensor.matmul(bias_ps[:], ones_row[:], bst[:],
                             start=True, stop=True)
            bias_b = persist.tile([128, C], F32)
            nc.vector.tensor_copy(bias_b[:], bias_ps[:])

        def wch(name, dc, cc=None):
            if cc is None:
                return w_sb[name][:, dc * C:(dc + 1) * C]
            return w_sb[name][:, dc * C + cc * 128: dc * C + (cc + 1) * 128]

        tpool = ctx.enter_context(tc.tile_pool(name="tstage", bufs=2))
        xpool = ctx.enter_context(tc.tile_pool(name="xstage", bufs=2))

        def emit_trans(rep, lw=None):
            # XBAR DMA transposes for rep's t/x inputs. Dispatched on the sync
            # queue; for rep>0 this happens BEFORE the previous rep's rs/myz
            # DMAs enter the queue, so next-rep inputs stage during attention.
            tT = [tpool.tile([128, TNS], BF16, name=f"tT{dc}_{rep}",
                             tag=f"tT{dc}") for dc in range(3)]
            nc.sync.dma_start_transpose(tT[0][:], tb_d[:, 0:128])
            nc.sync.dma_start_transpose(tT[1][:], tb_d[:, 128:256])
            if lw:
                lw("Wq", nc.sync)
                lw("Wk", nc.scalar)
            nc.sync.dma_start_transpose(tT[2][:], tb_d[:, 256:384])
            if lw:
                lw("Wv", nc.scalar)
            xTc = [[xpool.tile([128, 2048], BF16, name=f"xT{dc}_{hf}_{rep}",
                               tag=f"xT{dc}_{hf}") for hf in range(2)]
                   for dc in range(3)]
            for hf in range(2):
                for dc in range(3):
                    nc.sync.dma_start_transpose(
                        xTc[dc][hf][:],
                        xb_d[hf * 2048:(hf + 1) * 2048,
                             dc * 128:(dc + 1) * 128])
                    if lw and hf == 0 and dc == 0:
                        lw("Wp", nc.scalar)
            return tT, xTc

        trans = emit_trans(0, load_w)
        pend = None
        for rep in range(repeat):
            trans, pend = _one_pass(
                nc, tc, xb_d, tb_d, out_d, ones_col, hselA, hselB, wch,
                bias_b, dram, with_collective, rep, trans,
                emit_trans if rep + 1 < repeat else None, pend)
        pend()


def _one_pass(nc, tc, xb_d, tb_d, out_d, ones_col,
              hselA, hselB, wch, bias_b, dram, with_collective, rep,
              trans, emit_trans, pend_proj):
    qeng = [nc.sync, nc.sync]
    tT, xTc = trans
    with ExitStack() as octx:
        attin = octx.enter_context(tc.tile_pool(name="attin", bufs=1))
        # ---- qT ----
        qT = attin.tile([128, 3 * TNS], BF16, name="qT", tag="qT")
        with tc.tile_pool(name="qpsum", bufs=2, space="PSUM") as qpsum:
            for nt in range(TNS // 512):
                for cc in range(3):
                    ps = qpsum.tile([128, 512], F32, name="qps", tag="qps")
                    for dc in range(3):
                        nc.tensor.matmul(
                            ps[:], wch("Wq", dc, cc),
                            tT[dc][:, nt * 512:(nt + 1) * 512],
                            start=(dc == 0), stop=(dc == 2))
                    nc.scalar.copy(
                        qT[:, cc * TNS + nt * 512: cc * TNS + (nt + 1) * 512],
                        ps[:])

        # ---- kT, v ----
        kTc = [[attin.tile([128, 512], BF16, name=f"kT{cc}_{nt}",
                           tag=f"kT{cc}_{nt}") for nt in range(N // 512)]
               for cc in range(3)]
        v_n = [attin.tile([128, C], BF16, name=f"v{n32}", tag=f"v{n32}")
               for n32 in range(32)]
        with tc.tile_pool(name="kvpsum", bufs=3, space="PSUM") as kvpsum:
            for nt in range(N // 512):
                hf, xo = nt // 4, (nt % 4) * 512
                for cc in range(3):
                    ps = kvpsum.tile([128, 512], F32, name="kps", tag="kps")
                    for dc in range(3):
                        nc.tensor.matmul(
                            ps[:], wch("Wk", dc, cc),
                            xTc[dc][hf][:, xo:xo + 512],
                            start=(dc == 0), stop=(dc == 2))
                    nc.scalar.copy(kTc[cc][nt][:], ps[:])
                for j in range(4):
                    n32 = nt * 4 + j
                    ps = kvpsum.tile([128, C], F32, name="vps", tag="vps")
                    for dc in range(3):
                        nc.tensor.matmul(
                            ps[:],
                            xTc[dc][hf][:, xo + j * 128: xo + (j + 1) * 128],
                            wch("Wv", dc),
                            start=(dc == 0), stop=(dc == 2))
                    nc.scalar.copy(v_n[n32][:], ps[:])

        # previous rep's projection: emitted here so its wait on the last
        # ReduceScatter hides under this rep's q/k/v matmuls
        if pend_proj is not None:
            pend_proj()
        # next rep's input transposes: dispatch before this rep's rs/myz DMAs
        next_trans = emit_trans(rep + 1) if emit_trans is not None else None

        # ---- attention + pipelined pair exchange ----
        # myz assembles this core's [H, TN] slab of the permuted o^T.
        myz = dram.tile([H, TN], BF16, name=f"myz{rep}", tag="myz", bufs=2)
        with tc.tile_pool(name="spsum", bufs=3, space="PSUM") as spsum, \
             tc.tile_pool(name="opsum", bufs=1, space="PSUM") as opsum, \
             tc.tile_pool(name="dpsum", bufs=1, space="PSUM") as dpsum, \
             tc.tile_pool(name="epool", bufs=6) as epool, \
             tc.tile_pool(name="npool", bufs=2) as npool, \
             tc.tile_pool(name="mpool", bufs=6) as mpool, \
             tc.tile_pool(name="rsdram", bufs=1, space="DRAM") as rsdram:
            q0 = 0
            for T, qw in enumerate(CHUNKS):
                o_ps = [opsum.tile([128, 512], F32, name=f"ops{cc}",
                                   tag=f"ops{cc}")[:, 0:qw] for cc in range(3)]
                d_ps = dpsum.tile([1, 512], F32, name="dps", tag="dps")[:, 0:qw]
                for n32 in range(32):
                    s_ps = spsum.tile([128, 512], F32, name="sps",
                                      tag="sps")[:, 0:qw]
                    for cc in range(3):
                        nc.tensor.matmul(
                            s_ps,
                            kTc[cc][n32 // 4][:, (n32 % 4) * 128:
                                              (n32 % 4 + 1) * 128],
                            qT[:, cc * TNS + q0: cc * TNS + q0 + qw],
                            start=(cc == 0), stop=(cc == 2))
                    e_t = epool.tile([128, 512], BF16, name="e_t",
                                     tag="e_t")[:, 0:qw]
                    nc.scalar.activation(e_t, s_ps, EXP, scale=SCALE)
                    for cc in range(3):
                        nc.tensor.matmul(
                            o_ps[cc], v_n[n32][:, cc * 128:(cc + 1) * 128],
                            e_t, start=(n32 == 0), stop=(n32 == 31))
                    nc.tensor.matmul(d_ps, ones_col[:], e_t,
                                     start=(n32 == 0), stop=(n32 == 31))
                rec = npool.tile([1, 512], BF16, name="rec", tag="rec")[:, 0:qw]
                with nc.allow_low_precision(reason="1/D in bf16: 2^-9 ok"):
                    nc.vector.reciprocal(rec, d_ps)
                # masked reciprocal broadcasts: (1-h)/D and h/D
                bA_ps = spsum.tile([128, 512], F32, name="bAps",
                                   tag="sps")[:, 0:qw]
                nc.tensor.matmul(bA_ps, hselA[:], rec, start=True, stop=True)
                recA = npool.tile([128, 512], F32, name="recA",
                                  tag="recA")[:, 0:qw]
                nc.scalar.copy(recA, bA_ps)
                bB_ps = spsum.tile([128, 512], F32, name="bBps",
                                   tag="sps")[:, 0:qw]
                nc.tensor.matmul(bB_ps, hselB[:], rec, start=True, stop=True)
                recB = npool.tile([128, 512], F32, name="recB",
                                  tag="recB")[:, 0:qw]
                nc.scalar.copy(recB, bB_ps)

                rs_in = rsdram.tile([2 * H, 2 * qw], BF16, name=f"rsin{T}",
                                    tag=f"rsin{qw}", bufs=2)
                for cc in range(3):
                    m_t = mpool.tile([128, 1024], BF16, name="m_t",
                                     tag="m_t")[:, 0:2 * qw]
                    nc.vector.tensor_mul(m_t[:, 0:qw], o_ps[cc], recA)
                    nc.vector.tensor_mul(m_t[:, qw:2 * qw], o_ps[cc], recB)
                    nc.sync.dma_start(rs_in[cc * 128:(cc + 1) * 128, :], m_t)

                if with_collective:
                    ro = rsdram.tile([H, 2 * qw], BF16, name=f"rsout{T}",
                                     tag=f"rsout{qw}", bufs=2)
                    nc.gpsimd.collective_compute(
                        "ReduceScatter", mybir.AluOpType.add,
                        replica_groups=[[0, 1], [2, 3], [4, 5], [6, 7]],
                        ins=[rs_in[:].opt()], outs=[ro[:].opt()])
                    nc.sync.dma_start(myz[:, q0:q0 + qw], ro[:, 0:qw])
                    nc.sync.dma_start(myz[:, TNS + q0: TNS + q0 + qw],
                                      ro[:, qw:2 * qw])
                else:
                    # debug path: pretend partner's half equals ours
                    nc.sync.dma_start(myz[:, q0:q0 + qw], rs_in[0:H, 0:qw])
                    nc.sync.dma_start(myz[:, TNS + q0: TNS + q0 + qw],
                                      rs_in[0:H, qw:2 * qw])
                q0 += qw

    # ---- permuted output projection (own half only: TNS rows) ----
    # myz.flat viewed as [TNS, C] IS this core's slice of the permuted o;
    # XBAR-transpose it back to channel-major and hit it with Wp.
    # Returned as a closure; the caller emits it under the NEXT rep's q/k/v
    # so the wait on the last ReduceScatter is hidden.
    def emit_proj():
        zr = myz[:].rearrange("a b -> (a b)").rearrange("(r c) -> r c", c=C)
        with tc.tile_pool(name="fpool", bufs=4) as fpool, \
             tc.tile_pool(name="rtpool", bufs=2) as rtpool, \
             tc.tile_pool(name="fpsum", bufs=4, space="PSUM") as fpsum:
            rTc = [[rtpool.tile([128, 1024], BF16, name=f"rT{jc}_{hf}",
                                tag=f"rT{jc}_{hf}") for hf in range(2)]
                   for jc in range(3)]
            qi = 0
            for hf in range(2):
                for jc in range(3):
                    qeng[qi % 2].dma_start_transpose(
                        rTc[jc][hf][:],
                        zr[hf * 1024:(hf + 1) * 1024,
                           jc * 128:(jc + 1) * 128])
                    qi += 1
            for it in range(TNS // 128):
                hf, ri = it // 8, it % 8
                out_ps = fpsum.tile([128, C], F32, name="out_ps", tag="out_ps")
                for jc in range(3):
                    nc.tensor.matmul(
                        out_ps[:], rTc[jc][hf][:, ri * 128:(ri + 1) * 128],
                        wch("Wp", jc), start=(jc == 0), stop=(jc == 2))
                o_t = fpool.tile([128, C], F32, name="o_t", tag="o_t")
                nc.vector.tensor_add(o_t[:], out_ps[:], bias_b[:])
                nc.sync.dma_start(out_d[it * 128:(it + 1) * 128, :], o_t[:])
    return next_trans, emit_proj


def make_in_maps(inputs):
    import ml_dtypes
    x = np.asarray(inputs["x"], ml_dtypes.bfloat16)
    t = np.asarray(inputs["t"], ml_dtypes.bfloat16)
    ws = {n: np.asarray(inputs[n], ml_dtypes.bfloat16)
          for n in ("Wq", "Wk", "Wv", "Wp")}
    maps = []
    for p in range(N_CORES):
        b, h = p // 2, p % 2
        maps.append({
            "xb": np.ascontiguousarray(x[b]),
            "tb": np.ascontiguousarray(t[b, h * TNS:(h + 1) * TNS]),
            **ws,
            "bp": np.asarray(inputs["bp"], np.float32).reshape(1, C),
            "hselA": np.full((1, 128), 1.0 - h, ml_dtypes.bfloat16),
            "hselB": np.full((1, 128), float(h), ml_dtypes.bfloat16),
        })
    return maps


def assemble(results):
    out = np.empty((B, TN, C), np.float32)
    for p in range(N_CORES):
        b, h = p // 2, p % 2
        out[b, h * TNS:(h + 1) * TNS] = results[p]["out"]
    return out


_NC_CACHE = {}


def _get_nc(repeat=1):
    key = repeat
    if key not in _NC_CACHE:
        _NC_CACHE[key] = build(repeat=repeat)
    return _NC_CACHE[key]


def kernel(**inputs) -> np.ndarray:
    nc = _get_nc()
    in_maps = make_in_maps(inputs)
    res = run_bass_kernel_spmd(nc, in_maps, list(range(N_CORES)))
    return assemble(res.results)


# revision 20
# speedup vs baseline: 1.9475x; 1.1172x over previous
"""nn_CrossAttention Trainium2 kernel — 8-core SPMD Bass/Tile implementation.

Sharding: core p -> batch b = p//2, query-row half h = p%2 (data parallel over
B=4, sequence-parallel over TN within each batch pair).

Per-core dataflow (v4 — bf16 datapath, XBAR DMA transposes everywhere,
pipelined ReduceScatter exchange, small-tail chunk schedule):
  x/t/weights are pre-cast to bf16 on the host; tT/xT land channel-major in
  SBUF via hardware DMA-transpose issued in 512-row slices alternating
  between the two HWDGE queues (sync/scalar), so PE projections start as
  soon as the first slices arrive.
  qT,kT   projections in channel-major layout (bf16)
  v       projection in natural row-major layout (bf16)
  sT      score tiles computed *transposed* (keys on partitions, queries free)
  eT      exp(SCALE*s) straight from PSUM via ScalarE, cast to bf16
  D       softmax denominators via ones-vector matmul (PE partition reduce)

  Exchange: the reference's "transpose(1,2).reshape" permutation sends
  channels [192h,192h+192) x all TN queries to pair-member h. Each core
  writes its normalized o tiles into a [384, 2*qw] bf16 buffer twice —
  columns [0:qw] scaled by (1-h)/D and [qw:2qw] by h/D (h delivered as a
  per-core 0/1 input folded into the reciprocal-broadcast matmul) — and a
  pairwise ReduceScatter(add) then yields exactly the [192, 2*qw] slab this
  core needs: zeros from my masked half + partner's data. Chunked per
  query tile ([512,512,512,256,256]) so the collectives overlap the
  attention compute and the last exposed collective is small.

  out     each core projects only its own TN/2 output rows; the permuted
          rows are contiguous in the assembled [192, TN] buffer, which is
          re-transposed channel-major by XBAR DMA and hit with plain
          matmuls against Wp (+bp).
"""
from contextlib import ExitStack

import numpy as np

import concourse.bass as bass
import concourse.tile as tile
from concourse import bacc, mybir
from concourse.bass_utils import run_bass_kernel_spmd
from concourse.masks import make_identity

F32 = mybir.dt.float32
BF16 = mybir.dt.bfloat16
EXP = mybir.ActivationFunctionType.Exp

B, N, TN, C = 4, 4096, 4096, 384
TNS = TN // 2
H = C // 2          # channels per pair-member after the permutation split
SCALE = (C // 8) ** -0.5
N_CORES = 8
CHUNKS = (512, 512, 512, 256, 256)  # small tail hides the RS
assert sum(CHUNKS) == TNS


def build(repeat=1, with_collective=True):
    nc = bacc.Bacc("TRN2", target_bir_lowering=False, debug=False,
                   num_devices=N_CORES)
    xb_d = nc.dram_tensor("xb", [N, C], BF16, kind="ExternalInput").ap()
    tb_d = nc.dram_tensor("tb", [TNS, C], BF16, kind="ExternalInput").ap()
    w_d = {n: nc.dram_tensor(n, [C, C], BF16, kind="ExternalInput").ap()
           for n in ("Wq", "Wk", "Wv", "Wp")}
    bp_d = nc.dram_tensor("bp", [1, C], F32, kind="ExternalInput").ap()
    hA_d = nc.dram_tensor("hselA", [1, 128], BF16, kind="ExternalInput").ap()
    hB_d = nc.dram_tensor("hselB", [1, 128], BF16, kind="ExternalInput").ap()
    out_d = nc.dram_tensor("out", [TNS, C], F32, kind="ExternalOutput").ap()

    with tile.TileContext(nc) as tc:
        _kernel_body(nc, tc, xb_d, tb_d, w_d, bp_d, hA_d, hB_d, out_d,
                     repeat, with_collective)
    nc.compile()
    return nc


def _kernel_body(nc, tc, xb_d, tb_d, w_d, bp_d, hA_d, hB_d, out_d,
                 repeat, with_collective):
    with ExitStack() as ctx:
        consts = ctx.enter_context(tc.tile_pool(name="consts", bufs=1))
        persist = ctx.enter_context(tc.tile_pool(name="persist", bufs=1))
        dram = ctx.enter_context(tc.tile_pool(name="dram", bufs=1, space="DRAM"))

        w_sb = {}
        for name in ("Wq", "Wk", "Wv", "Wp"):
            w_sb[name] = persist.tile([128, 3 * C], BF16, name=f"{name}_sb",
                                      tag=f"{name}_sb")

        def load_w(name, eng):
            for dc in range(3):
                eng.dma_start(w_sb[name][:, dc * C:(dc + 1) * C],
                              w_d[name][dc * 128:(dc + 1) * 128, :])

        ones_col = consts.tile([128, 1], BF16)
        nc.vector.memset(ones_col[:], 1.0)
        ones_row = consts.tile([1, 128], F32)
        nc.vector.memset(ones_row[:], 1.0)
        hselA = consts.tile([1, 128], BF16)
        nc.scalar.dma_start(hselA[:], hA_d[:])
        hselB = consts.tile([1, 128], BF16)
        nc.scalar.dma_start(hselB[:], hB_d[:])

        bst = consts.tile([1, C], F32)
        nc.sync.dma_start(bst[:], bp_d[:])
        with tc.tile_pool(name="bpsum", bufs=1, space="PSUM") as bpsum:
            bias_ps = bpsum.tile([128, C], F32)
            nc.tensor.matmul(bias_ps[:], ones_row[:], bst[:],
                             start=True, stop=True)
            bias_b = persist.tile([128, C], F32)
            nc.vector.tensor_copy(bias_b[:], bias_ps[:])

        def wch(name, dc, cc=None):
            if cc is None:
                return w_sb[name][:, dc * C:(dc + 1) * C]
            return w_sb[name][:, dc * C + cc * 128: dc * C + (cc + 1) * 128]

        tpool = ctx.enter_context(tc.tile_pool(name="tstage", bufs=2))
        xpool = ctx.enter_context(tc.tile_pool(name="xstage", bufs=2))

        def emit_trans(rep, lw=None):
            # XBAR DMA transposes for rep's t/x inputs. Dispatched on the sync
            # queue; for rep>0 this happens BEFORE the previous rep's rs/myz
            # DMAs enter the queue, so next-rep inputs stage during attention.
            tT = [tpool.tile([128, TNS], BF16, name=f"tT{dc}_{rep}",
                             tag=f"tT{dc}") for dc in range(3)]
            nc.sync.dma_start_transpose(tT[0][:], tb_d[:, 0:128])
            nc.sync.dma_start_transpose(tT[1][:], tb_d[:, 128:256])
            if lw:
                lw("Wq", nc.sync)
                lw("Wk", nc.scalar)
            nc.sync.dma_start_transpose(tT[2][:], tb_d[:, 256:384])
            if lw:
                lw("Wv", nc.scalar)
            xTc = [[xpool.tile([128, 2048], BF16, name=f"xT{dc}_{hf}_{rep}",
                               tag=f"xT{dc}_{hf}") for hf in range(2)]
                   for dc in range(3)]
            for hf in range(2):
                for dc in range(3):
                    nc.sync.dma_start_transpose(
                        xTc[dc][hf][:],
                        xb_d[hf * 2048:(hf + 1) * 2048,
                             dc * 128:(dc + 1) * 128])
                    if lw and hf == 0 and dc == 0:
                        lw("Wp", nc.scalar)
            return tT, xTc

        trans = emit_trans(0, load_w)
        pend = None
        for rep in range(repeat):
            trans, pend = _one_pass(
                nc, tc, xb_d, tb_d, out_d, ones_col, hselA, hselB, wch,
                bias_b, dram, with_collective, rep, trans,
                emit_trans if rep + 1 < repeat else None, pend)
        pend()


def _one_pass(nc, tc, xb_d, tb_d, out_d, ones_col,
              hselA, hselB, wch, bias_b, dram, with_collective, rep,
              trans, emit_trans, pend_proj):
    qeng = [nc.sync, nc.sync]
    tT, xTc = trans
    with ExitStack() as octx:
        attin = octx.enter_context(tc.tile_pool(name="attin", bufs=1))
        # ---- qT ----
        qT = attin.tile([128, 3 * TNS], BF16, name="qT", tag="qT")
        with tc.tile_pool(name="qpsum", bufs=2, space="PSUM") as qpsum:
            for nt in range(TNS // 512):
                for cc in range(3):
                    ps = qpsum.tile([128, 512], F32, name="qps", tag="qps")
                    for dc in range(3):
                        nc.tensor.matmul(
                            ps[:], wch("Wq", dc, cc),
                            tT[dc][:, nt * 512:(nt + 1) * 512],
                            start=(dc == 0), stop=(dc == 2))
                    nc.scalar.copy(
                        qT[:, cc * TNS + nt * 512: cc * TNS + (nt + 1) * 512],
                        ps[:])

        # ---- kT, v ----
        kTc = [[attin.tile([128, 512], BF16, name=f"kT{cc}_{nt}",
                           tag=f"kT{cc}_{nt}") for nt in range(N // 512)]
               for cc in range(3)]
        v_n = [attin.tile([128, C], BF16, name=f"v{n32}", tag=f"v{n32}")
               for n32 in range(32)]
        with tc.tile_pool(name="kvpsum", bufs=3, space="PSUM") as kvpsum:
            for nt in range(N // 512):
                hf, xo = nt // 4, (nt % 4) * 512
                for cc in range(3):
                    ps = kvpsum.tile([128, 512], F32, name="kps", tag="kps")
                    for dc in range(3):
                        nc.tensor.matmul(
                            ps[:], wch("Wk", dc, cc),
                            xTc[dc][hf][:, xo:xo + 512],
                            start=(dc == 0), stop=(dc == 2))
                    nc.scalar.copy(kTc[cc][nt][:], ps[:])
                for j in range(4):
                    n32 = nt * 4 + j
                    ps = kvpsum.tile([128, C], F32, name="vps", tag="vps")
                    for dc in range(3):
                        nc.tensor.matmul(
                            ps[:],
                            xTc[dc][hf][:, xo + j * 128: xo + (j + 1) * 128],
                            wch("Wv", dc),
                            start=(dc == 0), stop=(dc == 2))
                    nc.scalar.copy(v_n[n32][:], ps[:])

        # previous rep's projection: emitted here so its wait on the last
        # ReduceScatter hides under this rep's q/k/v matmuls
        if pend_proj is not None:
            pend_proj()
        # next rep's input transposes: dispatch before this rep's rs/myz DMAs
        next_trans = emit_trans(rep + 1) if emit_trans is not None else None

        # ---- attention + pipelined pair exchange ----
        # myz assembles this core's [H, TN] slab of the permuted o^T.
        myz = dram.tile([H, TN], BF16, name=f"myz{rep}", tag="myz", bufs=2)
        with tc.tile_pool(name="spsum", bufs=3, space="PSUM") as spsum, \
             tc.tile_pool(name="opsum", bufs=1, space="PSUM") as opsum, \
             tc.tile_pool(name="dpsum", bufs=1, space="PSUM") as dpsum, \
             tc.tile_pool(name="epool", bufs=6) as epool, \
             tc.tile_pool(name="npool", bufs=2) as npool, \
             tc.tile_pool(name="mpool", bufs=6) as mpool, \
             tc.tile_pool(name="rsdram", bufs=1, space="DRAM") as rsdram:
            q0 = 0
            for T, qw in enumerate(CHUNKS):
                o_ps = [opsum.tile([128, 512], F32, name=f"ops{cc}",
                                   tag=f"ops{cc}")[:, 0:qw] for cc in range(3)]
                d_ps = dpsum.tile([1, 512], F32, name="dps", tag="dps")[:, 0:qw]
                def emit_o(e_prev, k32):
                    for cc in range(3):
                        nc.tensor.matmul(
                            o_ps[cc], v_n[k32][:, cc * 128:(cc + 1) * 128],
                            e_prev, start=(k32 == 0), stop=(k32 == 31))
                    nc.tensor.matmul(d_ps, ones_col[:], e_prev,
                                     start=(k32 == 0), stop=(k32 == 31))

                e_prev = None
                for n32 in range(32):
                    s_ps = spsum.tile([128, 512], F32, name="sps",
                                      tag="sps")[:, 0:qw]
                    for cc in range(3):
                        nc.tensor.matmul(
                            s_ps,
                            kTc[cc][n32 // 4][:, (n32 % 4) * 128:
                                              (n32 % 4 + 1) * 128],
                            qT[:, cc * TNS + q0: cc * TNS + q0 + qw],
                            start=(cc == 0), stop=(cc == 2))
                    e_t = epool.tile([128, 512], BF16, name="e_t",
                                     tag="e_t")[:, 0:qw]
                    nc.scalar.activation(e_t, s_ps, EXP, scale=SCALE)
                    # o-matmuls run one key-block behind so exp never
                    # stalls the PE
                    if e_prev is not None:
                        emit_o(e_prev, n32 - 1)
                    e_prev = e_t
                emit_o(e_prev, 31)
                rec = npool.tile([1, 512], BF16, name="rec", tag="rec")[:, 0:qw]
                with nc.allow_low_precision(reason="1/D in bf16: 2^-9 ok"):
                    nc.vector.reciprocal(rec, d_ps)
                # masked reciprocal broadcasts: (1-h)/D and h/D
                bA_ps = spsum.tile([128, 512], F32, name="bAps",
                                   tag="sps")[:, 0:qw]
                nc.tensor.matmul(bA_ps, hselA[:], rec, start=True, stop=True)
                recA = npool.tile([128, 512], F32, name="recA",
                                  tag="recA")[:, 0:qw]
                nc.scalar.copy(recA, bA_ps)
                bB_ps = spsum.tile([128, 512], F32, name="bBps",
                                   tag="sps")[:, 0:qw]
                nc.tensor.matmul(bB_ps, hselB[:], rec, start=True, stop=True)
                recB = npool.tile([128, 512], F32, name="recB",
                                  tag="recB")[:, 0:qw]
                nc.scalar.copy(recB, bB_ps)

                rs_in = rsdram.tile([2 * H, 2 * qw], BF16, name=f"rsin{T}",
                                    tag=f"rsin{qw}", bufs=2)
                for cc in range(3):
                    m_t = mpool.tile([128, 1024], BF16, name="m_t",
                                     tag="m_t")[:, 0:2 * qw]
                    nc.vector.tensor_mul(m_t[:, 0:qw], o_ps[cc], recA)
                    nc.vector.tensor_mul(m_t[:, qw:2 * qw], o_ps[cc], recB)
                    nc.sync.dma_start(rs_in[cc * 128:(cc + 1) * 128, :], m_t)

                if with_collective:
                    ro = rsdram.tile([H, 2 * qw], BF16, name=f"rsout{T}",
                                     tag=f"rsout{qw}", bufs=2)
                    nc.gpsimd.collective_compute(
                        "ReduceScatter", mybir.AluOpType.add,
                        replica_groups=[[0, 1], [2, 3], [4, 5], [6, 7]],
                        ins=[rs_in[:].opt()], outs=[ro[:].opt()])
                    nc.sync.dma_start(myz[:, q0:q0 + qw], ro[:, 0:qw])
                    nc.sync.dma_start(myz[:, TNS + q0: TNS + q0 + qw],
                                      ro[:, qw:2 * qw])
                else:
                    # debug path: pretend partner's half equals ours
                    nc.sync.dma_start(myz[:, q0:q0 + qw], rs_in[0:H, 0:qw])
                    nc.sync.dma_start(myz[:, TNS + q0: TNS + q0 + qw],
                                      rs_in[0:H, qw:2 * qw])
                q0 += qw

    # ---- permuted output projection (own half only: TNS rows) ----
    # myz.flat viewed as [TNS, C] IS this core's slice of the permuted o;
    # XBAR-transpose it back to channel-major and hit it with Wp.
    # Returned as a closure; the caller emits it under the NEXT rep's q/k/v
    # so the wait on the last ReduceScatter is hidden.
    def emit_proj():
        zr = myz[:].rearrange("a b -> (a b)").rearrange("(r c) -> r c", c=C)
        with tc.tile_pool(name="fpool", bufs=4) as fpool, \
             tc.tile_pool(name="rtpool", bufs=2) as rtpool, \
             tc.tile_pool(name="fpsum", bufs=4, space="PSUM") as fpsum:
            rTc = [[rtpool.tile([128, 1024], BF16, name=f"rT{jc}_{hf}",
                                tag=f"rT{jc}_{hf}") for hf in range(2)]
                   for jc in range(3)]
            qi = 0
            for hf in range(2):
                for jc in range(3):
                    qeng[qi % 2].dma_start_transpose(
                        rTc[jc][hf][:],
                        zr[hf * 1024:(hf + 1) * 1024,
                           jc * 128:(jc + 1) * 128])
                    qi += 1
            for it in range(TNS // 128):
                hf, ri = it // 8, it % 8
                out_ps = fpsum.tile([128, C], F32, name="out_ps", tag="out_ps")
                for jc in range(3):
                    nc.tensor.matmul(
                        out_ps[:], rTc[jc][hf][:, ri * 128:(ri + 1) * 128],
                        wch("Wp", jc), start=(jc == 0), stop=(jc == 2))
                o_t = fpool.tile([128, C], F32, name="o_t", tag="o_t")
                nc.vector.tensor_add(o_t[:], out_ps[:], bias_b[:])
                nc.sync.dma_start(out_d[it * 128:(it + 1) * 128, :], o_t[:])
    return next_trans, emit_proj


def make_in_maps(inputs):
    import ml_dtypes
    x = np.asarray(inputs["x"], ml_dtypes.bfloat16)
    t = np.asarray(inputs["t"], ml_dtypes.bfloat16)
    ws = {n: np.asarray(inputs[n], ml_dtypes.bfloat16)
          for n in ("Wq", "Wk", "Wv", "Wp")}
    maps = []
    for p in range(N_CORES):
        b, h = p // 2, p % 2
        maps.append({
            "xb": np.ascontiguousarray(x[b]),
            "tb": np.ascontiguousarray(t[b, h * TNS:(h + 1) * TNS]),
            **ws,
            "bp": np.asarray(inputs["bp"], np.float32).reshape(1, C),
            "hselA": np.full((1, 128), 1.0 - h, ml_dtypes.bfloat16),
            "hselB": np.full((1, 128), float(h), ml_dtypes.bfloat16),
        })
    return maps


def assemble(results):
    out = np.empty((B, TN, C), np.float32)
    for p in range(N_CORES):
        b, h = p // 2, p % 2
        out[b, h * TNS:(h + 1) * TNS] = results[p]["out"]
    return out


_NC_CACHE = {}


def _get_nc(repeat=1):
    key = repeat
    if key not in _NC_CACHE:
        _NC_CACHE[key] = build(repeat=repeat)
    return _NC_CACHE[key]


def kernel(**inputs) -> np.ndarray:
    nc = _get_nc()
    in_maps = make_in_maps(inputs)
    res = run_bass_kernel_spmd(nc, in_maps, list(range(N_CORES)))
    return assemble(res.results)


# revision 21
# speedup vs baseline: 1.9680x; 1.0105x over previous
"""nn_CrossAttention Trainium2 kernel — 8-core SPMD Bass/Tile implementation.

Sharding: core p -> batch b = p//2, query-row half h = p%2 (data parallel over
B=4, sequence-parallel over TN within each batch pair).

Per-core dataflow (v4 — bf16 datapath, XBAR DMA transposes everywhere,
pipelined ReduceScatter exchange, small-tail chunk schedule):
  x/t/weights are pre-cast to bf16 on the host; tT/xT land channel-major in
  SBUF via hardware DMA-transpose issued in 512-row slices alternating
  between the two HWDGE queues (sync/scalar), so PE projections start as
  soon as the first slices arrive.
  qT,kT   projections in channel-major layout (bf16)
  v       projection in natural row-major layout (bf16)
  sT      score tiles computed *transposed* (keys on partitions, queries free)
  eT      exp(SCALE*s) straight from PSUM via ScalarE, cast to bf16
  D       softmax denominators via ones-vector matmul (PE partition reduce)

  Exchange: the reference's "transpose(1,2).reshape" permutation sends
  channels [192h,192h+192) x all TN queries to pair-member h. Each core
  writes its normalized o tiles into a [384, 2*qw] bf16 buffer twice —
  columns [0:qw] scaled by (1-h)/D and [qw:2qw] by h/D (h delivered as a
  per-core 0/1 input folded into the reciprocal-broadcast matmul) — and a
  pairwise ReduceScatter(add) then yields exactly the [192, 2*qw] slab this
  core needs: zeros from my masked half + partner's data. Chunked per
  query tile ([512,512,512,256,256]) so the collectives overlap the
  attention compute and the last exposed collective is small.

  out     each core projects only its own TN/2 output rows; the permuted
          rows are contiguous in the assembled [192, TN] buffer, which is
          re-transposed channel-major by XBAR DMA and hit with plain
          matmuls against Wp (+bp).
"""
from contextlib import ExitStack

import numpy as np

import concourse.bass as bass
import concourse.tile as tile
from concourse import bacc, mybir
from concourse.bass_utils import run_bass_kernel_spmd
from concourse.masks import make_identity

F32 = mybir.dt.float32
BF16 = mybir.dt.bfloat16
EXP = mybir.ActivationFunctionType.Exp

B, N, TN, C = 4, 4096, 4096, 384
TNS = TN // 2
H = C // 2          # channels per pair-member after the permutation split
SCALE = (C // 8) ** -0.5
N_CORES = 8
CHUNKS = (512, 512, 512, 512)  # even chunks: fewest collectives
assert sum(CHUNKS) == TNS


def build(repeat=1, with_collective=True):
    nc = bacc.Bacc("TRN2", target_bir_lowering=False, debug=False,
                   num_devices=N_CORES)
    xb_d = nc.dram_tensor("xb", [N, C], BF16, kind="ExternalInput").ap()
    tb_d = nc.dram_tensor("tb", [TNS, C], BF16, kind="ExternalInput").ap()
    w_d = {n: nc.dram_tensor(n, [C, C], BF16, kind="ExternalInput").ap()
           for n in ("Wq", "Wk", "Wv", "Wp")}
    bp_d = nc.dram_tensor("bp", [1, C], F32, kind="ExternalInput").ap()
    hA_d = nc.dram_tensor("hselA", [1, 128], BF16, kind="ExternalInput").ap()
    hB_d = nc.dram_tensor("hselB", [1, 128], BF16, kind="ExternalInput").ap()
    out_d = nc.dram_tensor("out", [TNS, C], F32, kind="ExternalOutput").ap()

    with tile.TileContext(nc) as tc:
        _kernel_body(nc, tc, xb_d, tb_d, w_d, bp_d, hA_d, hB_d, out_d,
                     repeat, with_collective)
    nc.compile()
    return nc


def _kernel_body(nc, tc, xb_d, tb_d, w_d, bp_d, hA_d, hB_d, out_d,
                 repeat, with_collective):
    with ExitStack() as ctx:
        consts = ctx.enter_context(tc.tile_pool(name="consts", bufs=1))
        persist = ctx.enter_context(tc.tile_pool(name="persist", bufs=1))
        dram = ctx.enter_context(tc.tile_pool(name="dram", bufs=1, space="DRAM"))

        w_sb = {}
        for name in ("Wq", "Wk", "Wv", "Wp"):
            w_sb[name] = persist.tile([128, 3 * C], BF16, name=f"{name}_sb",
                                      tag=f"{name}_sb")

        def load_w(name, eng):
            for dc in range(3):
                eng.dma_start(w_sb[name][:, dc * C:(dc + 1) * C],
                              w_d[name][dc * 128:(dc + 1) * 128, :])

        ones_col = consts.tile([128, 1], BF16)
        nc.vector.memset(ones_col[:], 1.0)
        ones_row = consts.tile([1, 128], F32)
        nc.vector.memset(ones_row[:], 1.0)
        hselA = consts.tile([1, 128], BF16)
        nc.scalar.dma_start(hselA[:], hA_d[:])
        hselB = consts.tile([1, 128], BF16)
        nc.scalar.dma_start(hselB[:], hB_d[:])

        bst = consts.tile([1, C], F32)
        nc.sync.dma_start(bst[:], bp_d[:])
        with tc.tile_pool(name="bpsum", bufs=1, space="PSUM") as bpsum:
            bias_ps = bpsum.tile([128, C], F32)
            nc.tensor.matmul(bias_ps[:], ones_row[:], bst[:],
                             start=True, stop=True)
            bias_b = persist.tile([128, C], F32)
            nc.vector.tensor_copy(bias_b[:], bias_ps[:])

        def wch(name, dc, cc=None):
            if cc is None:
                return w_sb[name][:, dc * C:(dc + 1) * C]
            return w_sb[name][:, dc * C + cc * 128: dc * C + (cc + 1) * 128]

        tpool = ctx.enter_context(tc.tile_pool(name="tstage", bufs=2))
        xpool = ctx.enter_context(tc.tile_pool(name="xstage", bufs=2))

        def emit_trans(rep, lw=None):
            # XBAR DMA transposes for rep's t/x inputs. Dispatched on the sync
            # queue; for rep>0 this happens BEFORE the previous rep's rs/myz
            # DMAs enter the queue, so next-rep inputs stage during attention.
            tT = [tpool.tile([128, TNS], BF16, name=f"tT{dc}_{rep}",
                             tag=f"tT{dc}") for dc in range(3)]
            nc.sync.dma_start_transpose(tT[0][:], tb_d[:, 0:128])
            nc.sync.dma_start_transpose(tT[1][:], tb_d[:, 128:256])
            if lw:
                lw("Wq", nc.sync)
                lw("Wk", nc.scalar)
            nc.sync.dma_start_transpose(tT[2][:], tb_d[:, 256:384])
            if lw:
                lw("Wv", nc.scalar)
            xTc = [[xpool.tile([128, 2048], BF16, name=f"xT{dc}_{hf}_{rep}",
                               tag=f"xT{dc}_{hf}") for hf in range(2)]
                   for dc in range(3)]
            for hf in range(2):
                for dc in range(3):
                    nc.sync.dma_start_transpose(
                        xTc[dc][hf][:],
                        xb_d[hf * 2048:(hf + 1) * 2048,
                             dc * 128:(dc + 1) * 128])
                    if lw and hf == 0 and dc == 0:
                        lw("Wp", nc.scalar)
            return tT, xTc

        trans = emit_trans(0, load_w)
        pend = None
        for rep in range(repeat):
            trans, pend = _one_pass(
                nc, tc, xb_d, tb_d, out_d, ones_col, hselA, hselB, wch,
                bias_b, dram, with_collective, rep, trans,
                emit_trans if rep + 1 < repeat else None, pend)
        pend()


def _one_pass(nc, tc, xb_d, tb_d, out_d, ones_col,
              hselA, hselB, wch, bias_b, dram, with_collective, rep,
              trans, emit_trans, pend_proj):
    qeng = [nc.sync, nc.sync]
    tT, xTc = trans
    with ExitStack() as octx:
        attin = octx.enter_context(tc.tile_pool(name="attin", bufs=1))
        # ---- qT ----
        qT = attin.tile([128, 3 * TNS], BF16, name="qT", tag="qT")
        with tc.tile_pool(name="qpsum", bufs=2, space="PSUM") as qpsum:
            for nt in range(TNS // 512):
                for cc in range(3):
                    ps = qpsum.tile([128, 512], F32, name="qps", tag="qps")
                    for dc in range(3):
                        nc.tensor.matmul(
                            ps[:], wch("Wq", dc, cc),
                            tT[dc][:, nt * 512:(nt + 1) * 512],
                            start=(dc == 0), stop=(dc == 2))
                    nc.scalar.copy(
                        qT[:, cc * TNS + nt * 512: cc * TNS + (nt + 1) * 512],
                        ps[:])

        # ---- kT, v ----
        kTc = [[attin.tile([128, 512], BF16, name=f"kT{cc}_{nt}",
                           tag=f"kT{cc}_{nt}") for nt in range(N // 512)]
               for cc in range(3)]
        v_n = [attin.tile([128, C], BF16, name=f"v{n32}", tag=f"v{n32}")
               for n32 in range(32)]
        with tc.tile_pool(name="kvpsum", bufs=3, space="PSUM") as kvpsum:
            for nt in range(N // 512):
                hf, xo = nt // 4, (nt % 4) * 512
                for cc in range(3):
                    ps = kvpsum.tile([128, 512], F32, name="kps", tag="kps")
                    for dc in range(3):
                        nc.tensor.matmul(
                            ps[:], wch("Wk", dc, cc),
                            xTc[dc][hf][:, xo:xo + 512],
                            start=(dc == 0), stop=(dc == 2))
                    nc.scalar.copy(kTc[cc][nt][:], ps[:])
                for j in range(4):
                    n32 = nt * 4 + j
                    ps = kvpsum.tile([128, C], F32, name="vps", tag="vps")
                    for dc in range(3):
                        nc.tensor.matmul(
                            ps[:],
                            xTc[dc][hf][:, xo + j * 128: xo + (j + 1) * 128],
                            wch("Wv", dc),
                            start=(dc == 0), stop=(dc == 2))
                    nc.scalar.copy(v_n[n32][:], ps[:])

        # previous rep's projection: emitted here so its wait on the last
        # ReduceScatter hides under this rep's q/k/v matmuls
        if pend_proj is not None:
            pend_proj()
        # next rep's input transposes: dispatch before this rep's rs/myz DMAs
        next_trans = emit_trans(rep + 1) if emit_trans is not None else None

        # ---- attention + pipelined pair exchange ----
        # myz assembles this core's [H, TN] slab of the permuted o^T.
        myz = dram.tile([H, TN], BF16, name=f"myz{rep}", tag="myz", bufs=2)
        with tc.tile_pool(name="spsum", bufs=3, space="PSUM") as spsum, \
             tc.tile_pool(name="opsum", bufs=1, space="PSUM") as opsum, \
             tc.tile_pool(name="dpsum", bufs=1, space="PSUM") as dpsum, \
             tc.tile_pool(name="epool", bufs=6) as epool, \
             tc.tile_pool(name="npool", bufs=2) as npool, \
             tc.tile_pool(name="mpool", bufs=6) as mpool, \
             tc.tile_pool(name="rsdram", bufs=1, space="DRAM") as rsdram:
            q0 = 0
            for T, qw in enumerate(CHUNKS):
                o_ps = [opsum.tile([128, 512], F32, name=f"ops{cc}",
                                   tag=f"ops{cc}")[:, 0:qw] for cc in range(3)]
                d_ps = dpsum.tile([1, 512], F32, name="dps", tag="dps")[:, 0:qw]
                def emit_o(e_prev, k32):
                    for cc in range(3):
                        nc.tensor.matmul(
                            o_ps[cc], v_n[k32][:, cc * 128:(cc + 1) * 128],
                            e_prev, start=(k32 == 0), stop=(k32 == 31))
                    nc.tensor.matmul(d_ps, ones_col[:], e_prev,
                                     start=(k32 == 0), stop=(k32 == 31))

                e_prev = None
                for n32 in range(32):
                    s_ps = spsum.tile([128, 512], F32, name="sps",
                                      tag="sps")[:, 0:qw]
                    for cc in range(3):
                        nc.tensor.matmul(
                            s_ps,
                            kTc[cc][n32 // 4][:, (n32 % 4) * 128:
                                              (n32 % 4 + 1) * 128],
                            qT[:, cc * TNS + q0: cc * TNS + q0 + qw],
                            start=(cc == 0), stop=(cc == 2))
                    e_t = epool.tile([128, 512], BF16, name="e_t",
                                     tag="e_t")[:, 0:qw]
                    nc.scalar.activation(e_t, s_ps, EXP, scale=SCALE)
                    # o-matmuls run one key-block behind so exp never
                    # stalls the PE
                    if e_prev is not None:
                        emit_o(e_prev, n32 - 1)
                    e_prev = e_t
                emit_o(e_prev, 31)
                rec = npool.tile([1, 512], BF16, name="rec", tag="rec")[:, 0:qw]
                with nc.allow_low_precision(reason="1/D in bf16: 2^-9 ok"):
                    nc.vector.reciprocal(rec, d_ps)
                # masked reciprocal broadcasts: (1-h)/D and h/D
                bA_ps = spsum.tile([128, 512], F32, name="bAps",
                                   tag="sps")[:, 0:qw]
                nc.tensor.matmul(bA_ps, hselA[:], rec, start=True, stop=True)
                recA = npool.tile([128, 512], F32, name="recA",
                                  tag="recA")[:, 0:qw]
                nc.scalar.copy(recA, bA_ps)
                bB_ps = spsum.tile([128, 512], F32, name="bBps",
                                   tag="sps")[:, 0:qw]
                nc.tensor.matmul(bB_ps, hselB[:], rec, start=True, stop=True)
                recB = npool.tile([128, 512], F32, name="recB",
                                  tag="recB")[:, 0:qw]
                nc.scalar.copy(recB, bB_ps)

                rs_in = rsdram.tile([2 * H, 2 * qw], BF16, name=f"rsin{T}",
                                    tag=f"rsin{qw}", bufs=2)
                for cc in range(3):
                    m_t = mpool.tile([128, 1024], BF16, name="m_t",
                                     tag="m_t")[:, 0:2 * qw]
                    nc.vector.tensor_mul(m_t[:, 0:qw], o_ps[cc], recA)
                    nc.vector.tensor_mul(m_t[:, qw:2 * qw], o_ps[cc], recB)
                    nc.sync.dma_start(rs_in[cc * 128:(cc + 1) * 128, :], m_t)

                if with_collective:
                    ro = rsdram.tile([H, 2 * qw], BF16, name=f"rsout{T}",
                                     tag=f"rsout{qw}", bufs=2)
                    nc.gpsimd.collective_compute(
                        "ReduceScatter", mybir.AluOpType.add,
                        replica_groups=[[0, 1], [2, 3], [4, 5], [6, 7]],
                        ins=[rs_in[:].opt()], outs=[ro[:].opt()])
                    nc.sync.dma_start(myz[:, q0:q0 + qw], ro[:, 0:qw])
                    nc.sync.dma_start(myz[:, TNS + q0: TNS + q0 + qw],
                                      ro[:, qw:2 * qw])
                else:
                    # debug path: pretend partner's half equals ours
                    nc.sync.dma_start(myz[:, q0:q0 + qw], rs_in[0:H, 0:qw])
                    nc.sync.dma_start(myz[:, TNS + q0: TNS + q0 + qw],
                                      rs_in[0:H, qw:2 * qw])
                q0 += qw

    # ---- permuted output projection (own half only: TNS rows) ----
    # myz.flat viewed as [TNS, C] IS this core's slice of the permuted o;
    # XBAR-transpose it back to channel-major and hit it with Wp.
    # Returned as a closure; the caller emits it under the NEXT rep's q/k/v
    # so the wait on the last ReduceScatter is hidden.
    def emit_proj():
        zr = myz[:].rearrange("a b -> (a b)").rearrange("(r c) -> r c", c=C)
        with tc.tile_pool(name="fpool", bufs=4) as fpool, \
             tc.tile_pool(name="rtpool", bufs=2) as rtpool, \
             tc.tile_pool(name="fpsum", bufs=4, space="PSUM") as fpsum:
            rTc = [[rtpool.tile([128, 1024], BF16, name=f"rT{jc}_{hf}",
                                tag=f"rT{jc}_{hf}") for hf in range(2)]
                   for jc in range(3)]
            qi = 0
            for hf in range(2):
                for jc in range(3):
                    qeng[qi % 2].dma_start_transpose(
                        rTc[jc][hf][:],
                        zr[hf * 1024:(hf + 1) * 1024,
                           jc * 128:(jc + 1) * 128])
                    qi += 1
            for it in range(TNS // 128):
                hf, ri = it // 8, it % 8
                out_ps = fpsum.tile([128, C], F32, name="out_ps", tag="out_ps")
                for jc in range(3):
                    nc.tensor.matmul(
                        out_ps[:], rTc[jc][hf][:, ri * 128:(ri + 1) * 128],
                        wch("Wp", jc), start=(jc == 0), stop=(jc == 2))
                o_t = fpool.tile([128, C], F32, name="o_t", tag="o_t")
                nc.vector.tensor_add(o_t[:], out_ps[:], bias_b[:])
                nc.sync.dma_start(out_d[it * 128:(it + 1) * 128, :], o_t[:])
    return next_trans, emit_proj


def make_in_maps(inputs):
    import ml_dtypes
    x = np.asarray(inputs["x"], ml_dtypes.bfloat16)
    t = np.asarray(inputs["t"], ml_dtypes.bfloat16)
    ws = {n: np.asarray(inputs[n], ml_dtypes.bfloat16)
          for n in ("Wq", "Wk", "Wv", "Wp")}
    maps = []
    for p in range(N_CORES):
        b, h = p // 2, p % 2
        maps.append({
            "xb": np.ascontiguousarray(x[b]),
            "tb": np.ascontiguousarray(t[b, h * TNS:(h + 1) * TNS]),
            **ws,
            "bp": np.asarray(inputs["bp"], np.float32).reshape(1, C),
            "hselA": np.full((1, 128), 1.0 - h, ml_dtypes.bfloat16),
            "hselB": np.full((1, 128), float(h), ml_dtypes.bfloat16),
        })
    return maps


def assemble(results):
    out = np.empty((B, TN, C), np.float32)
    for p in range(N_CORES):
        b, h = p // 2, p % 2
        out[b, h * TNS:(h + 1) * TNS] = results[p]["out"]
    return out


_NC_CACHE = {}


def _get_nc(repeat=1):
    key = repeat
    if key not in _NC_CACHE:
        _NC_CACHE[key] = build(repeat=repeat)
    return _NC_CACHE[key]


def kernel(**inputs) -> np.ndarray:
    nc = _get_nc()
    in_maps = make_in_maps(inputs)
    res = run_bass_kernel_spmd(nc, in_maps, list(range(N_CORES)))
    return assemble(res.results)
